# revision 57
# baseline (speedup 1.0000x reference)
"""Trainium2 Bass kernel for nn_LlamaAttention_kvcache (sparse H2O attention).

Strategy (8 NeuronCores, tensor-parallel over heads, 4 heads/core):
  Phase 1 (device): q/k projections (scale folded into Wq) in bf16, RoPE,
    causal-only QK^T (lower-triangular k-slices per q-tile), exp via the
    activation engine (row sums piggybacked on accum_out; diagonal slice
    masked post-exp with gpsimd affine_select), per-head softmax column
    scores accumulated with rank-1 (1/rowsum)^T @ E matmuls.
  Host: exact top-k selection per head (matches jax.lax.top_k tie-breaking),
    gathers kept k-columns / x-rows.
  Phase 2 (device): kept-column scores + eviction decomposition
      aw_new @ v = [M o (aw + 1e9)] @ v_kept  -  1e9 * (sum_all v)
    (evicted columns contribute exactly -1e9 * v); the +1e9*step mask is
    generated on-device from kept indices (iota + tensor_scalar is_ge);
    then row-parallel o_proj in bf16. Host sums the 8 per-core partials.
  Falls back to a general (mask-as-data, fp32) path when attention_mask is
  not the standard causal mask.
"""

import contextlib
import sys

for p in ("/opt/trn_rl_repo", "/root/.axon_site/_ro/trn_rl_repo"):
    if p not in sys.path:
        sys.path.append(p)

import ml_dtypes
import numpy as np

import concourse.bacc as bacc
import concourse.mybir as mybir
import concourse.tile as tile
from concourse.bass_utils import run_bass_kernel_spmd

F32 = mybir.dt.float32
F32R = mybir.dt.float32r
F16 = mybir.dt.float16
BF16 = mybir.dt.bfloat16
NPBF = ml_dtypes.bfloat16
P = 128
S = 2048
H = 4096
NH = 32
HD = 128
NCORES = 8
HPC = NH // NCORES          # heads per core = 4
KC = H // P                 # 32 k-chunks over the 4096 contraction
KEEP = int(0.1 * S)         # 204 top-k heavy hitters
NKEPT = KEEP + 2            # + last-2 local tokens = 206
T1 = NKEPT - P              # ragged second kept-tile rows = 78
KPAD = 256                  # padded kept count (general fallback only)
EXP = mybir.ActivationFunctionType.Exp

_cache = {}


def _run(nc, in_maps, core_ids):
    return run_bass_kernel_spmd(nc, in_maps, core_ids)


def _build_phase1():
    nc = bacc.Bacc("TRN2", target_bir_lowering=False, debug=False,
                   num_devices=NCORES)
    xb = nc.dram_tensor("xb", [H, S], BF16, kind="ExternalInput").ap()
    wq = nc.dram_tensor("wq", [H, HPC * HD], BF16, kind="ExternalInput").ap()
    wk = nc.dram_tensor("wk", [H, HPC * HD], BF16, kind="ExternalInput").ap()
    cosd = nc.dram_tensor("cos", [P, S], F16, kind="ExternalInput").ap()
    sind = nc.dram_tensor("sin", [P, S], F16, kind="ExternalInput").ap()
    identd = nc.dram_tensor("ident", [P, P], F16, kind="ExternalInput").ap()
    mtaild = nc.dram_tensor("mtail", [P, 4 * 512], F16, kind="ExternalInput").ap()
    scores_o = nc.dram_tensor("scores", [HPC, S], F32,
                              kind="ExternalOutput").ap()
    qt_o = nc.dram_tensor("qt", [HPC * HD, S], F16, kind="ExternalOutput").ap()
    kt_o = nc.dram_tensor("kt", [HPC * HD, S], F16, kind="ExternalOutput").ap()

    with tile.TileContext(nc) as tc, contextlib.ExitStack() as ctx:
        const = ctx.enter_context(tc.tile_pool(name="const", bufs=1))
        wpool = ctx.enter_context(tc.tile_pool(name="wpool", bufs=1))
        xpool = ctx.enter_context(tc.tile_pool(name="xpool", bufs=10))
        qkres = ctx.enter_context(tc.tile_pool(name="qkres", bufs=1))
        rpool = ctx.enter_context(tc.tile_pool(name="rpool", bufs=2))
        epool = ctx.enter_context(tc.tile_pool(name="epool", bufs=3))
        vpool = ctx.enter_context(tc.tile_pool(name="vpool", bufs=3))

        cos_sb = const.tile([P, S], F16, name="cos", tag="cos")
        sin_sb = const.tile([P, S], F16, name="sin", tag="sin")
        ident_sb = const.tile([P, P], F16, name="ident", tag="ident")
        mtail_sb = const.tile([P, 4 * 512], F16, name="mtail", tag="mtail")
        nc.gpsimd.dma_start(ident_sb[:], identd[:, :])
        nc.gpsimd.dma_start(mtail_sb[:], mtaild[:, :])

        # resident roped q/k per head, fp16: [128 d, 2048 s]
        qt_sb = [qkres.tile([P, S], F16, name=f"qt{h}", tag=f"qt{h}")
                 for h in range(HPC)]
        kt_sb = [qkres.tile([P, S], F16, name=f"kt{h}", tag=f"kt{h}")
                 for h in range(HPC)]

        # w tiles are DMA'd lazily inside the sq=0 loop (interleaved with x
        # chunks) so the first matmul isn't stuck behind an 8MB preamble
        wq_sb = [wpool.tile([P, HPC * HD], BF16, name=f"wq{kc}", tag=f"wq{kc}")
                 for kc in range(KC)]
        wk_sb = [wpool.tile([P, HPC * HD], BF16, name=f"wk{kc}", tag=f"wk{kc}")
                 for kc in range(KC)]

        def rope(ps, out_slice, ssl):
            m = rpool.tile([P, 512], F32, name="ropetmp", tag="ropetmp")
            nc.vector.tensor_mul(m[:], ps[:], cos_sb[:, ssl])
            rot = rpool.tile([P, 512], F32, name="roperot", tag="roperot")
            nc.vector.tensor_scalar_mul(rot[0:64, :], ps[64:128, :], -1.0)
            nc.vector.tensor_scalar_mul(rot[64:128, :], ps[0:64, :], 1.0)
            rs_ = rpool.tile([P, 512], F32, name="ropesin", tag="ropesin")
            nc.vector.tensor_mul(rs_[:], rot[:], sin_sb[:, ssl])
            nc.vector.tensor_add(out_slice, m[:], rs_[:])

        # two passes (q then k) per s-quarter with double-buffered PSUM so
        # the next pass's matmuls overlap this pass's RoPE drain; x chunks
        # are resident per quarter (one DMA, reused by both passes, with
        # per-tag rolling prefetch into the next quarter)
        with tc.tile_pool(name="ppool", bufs=2, space="PSUM") as ppool:
            for sq in range(4):          # 512-wide s quarters
                ssl = slice(sq * 512, (sq + 1) * 512)
                xc_sq = []
                for w_sb, wd, out_tiles, is_q in (
                        (wq_sb, wq, qt_sb, True), (wk_sb, wk, kt_sb, False)):
                    ps = [ppool.tile([P, 512], F32, name=f"pj{h}", tag=f"pj{h}")
                          for h in range(HPC)]
                    for kc in range(KC):
                        if is_q:
                            xc = xpool.tile([P, 512], BF16, name="xc",
                                            tag=f"xc{kc}", bufs=1)
                            nc.sync.dma_start(xc[:], xb[kc * P:(kc + 1) * P, ssl])
                            xc_sq.append(xc)
                        else:
                            xc = xc_sq[kc]
                        if sq == 0:
                            nc.scalar.dma_start(w_sb[kc][:],
                                                wd[kc * P:(kc + 1) * P, :])
                            if is_q and kc == 2:
                                nc.gpsimd.dma_start(cos_sb[:], cosd[:, :])
                                nc.gpsimd.dma_start(sin_sb[:], sind[:, :])
                        for h in range(HPC):
                            nc.tensor.matmul(
                                ps[h][:], lhsT=w_sb[kc][:, h * HD:(h + 1) * HD],
                                rhs=xc[:], start=(kc == 0), stop=(kc == KC - 1))
                    for h in range(HPC):
                        rope(ps[h], out_tiles[h][:, ssl], ssl)
        for h in range(HPC):
            nc.sync.dma_start(qt_o[h * HD:(h + 1) * HD, :], qt_sb[h][:, :])
            nc.sync.dma_start(kt_o[h * HD:(h + 1) * HD, :], kt_sb[h][:, :])

        spool = ctx.enter_context(tc.tile_pool(name="spool", bufs=1, space="PSUM"))
        apool = ctx.enter_context(tc.tile_pool(name="apool", bufs=2, space="PSUM"))

        # causal attention scores per head. The strictly-future entries of
        # the diagonal 512-slice get -60000 added in PSUM via an
        # identity-weight matmul (exp then yields exact zeros), so every
        # activation is uniform with accum_out and slices pair into
        # 1024-wide activations. Score matmuls write each 512-slice to its
        # own partition of a [4, 512] one-bank accumulator and lag 2
        # q-tiles so the PE never waits on the exp/rowsum chain.
        for h in range(HPC):
            sc_ps = spool.tile([1, S], F32, name="scps", tag="scps")
            pend = []

            def flush_one():
                oqt, oE, orb, onsl = pend.pop(0)
                for s in range(onsl):
                    ksl = slice(s * 512, (s + 1) * 512)
                    nc.tensor.matmul(sc_ps[:, ksl], lhsT=orb[:],
                                     rhs=oE[:, ksl],
                                     start=(oqt == 4 * s), stop=(oqt == 15))

            for qt in range(16):
                qsl = slice(qt * P, (qt + 1) * P)
                nsl = (qt + 1 + 3) // 4      # 512-slices overlapping [0,(qt+1)*128)
                m = qt % 4                   # within-slice diagonal offset
                E = epool.tile([P, S], BF16, name="E", tag="E")
                for pr in range((nsl + 1) // 2):
                    s0 = 2 * pr
                    w = 1024 if s0 + 1 < nsl else 512
                    aw = apool.tile([P, 1024], F32, name="aw", tag="aw")
                    for k in range(w // 512):
                        s = s0 + k
                        ksl = slice(s * 512, (s + 1) * 512)
                        awsl = aw[:, k * 512:(k + 1) * 512]
                        if s == nsl - 1:
                            nc.tensor.matmul(awsl, lhsT=qt_sb[h][:, qsl],
                                             rhs=kt_sb[h][:, ksl],
                                             start=True, stop=False)
                            nc.tensor.matmul(
                                awsl, lhsT=ident_sb[:],
                                rhs=mtail_sb[:, m * 512:(m + 1) * 512],
                                start=False, stop=True)
                        else:
                            nc.tensor.matmul(awsl, lhsT=qt_sb[h][:, qsl],
                                             rhs=kt_sb[h][:, ksl],
                                             start=True, stop=True)
                    nc.scalar.activation(E[:, s0 * 512: s0 * 512 + w],
                                         aw[:, 0:w], EXP)
                racc = vpool.tile([P, 1], F32, name="racc", tag="racc")
                nc.vector.tensor_reduce(racc[:], E[:, 0:nsl * 512],
                                        axis=mybir.AxisListType.X,
                                        op=mybir.AluOpType.add)
                rinv = vpool.tile([P, 1], F32, name="rinv", tag="rinv")
                nc.vector.reciprocal(rinv[:], racc[:])
                rb = vpool.tile([P, 1], BF16, name="rb", tag="rb")
                nc.vector.tensor_copy(rb[:], rinv[:])
                pend.append((qt, E, rb, nsl))
                if len(pend) > 2:
                    flush_one()
            while pend:
                flush_one()
            scsb = vpool.tile([1, S], F32, name="scsb", tag="scsb", bufs=2)
            nc.vector.tensor_copy(scsb[:], sc_ps[:])
            nc.sync.dma_start(scores_o[h:h + 1, :], scsb[:])
    nc.compile()
    return nc


def _build_phase2():
    nc = bacc.Bacc("TRN2", target_bir_lowering=False, debug=False,
                   num_devices=NCORES)
    qtd = nc.dram_tensor("qt2", [HPC * HD, S], F16, kind="ExternalInput").ap()
    ktk = nc.dram_tensor("ktk", [HPC * HD, NKEPT], F16, kind="ExternalInput").ap()
    keptq = nc.dram_tensor("keptq", [P, HPC * 2], F32, kind="ExternalInput").ap()
    xtk = nc.dram_tensor("xtk", [H, HPC * NKEPT], BF16, kind="ExternalInput").ap()
    wv = nc.dram_tensor("wv", [H, HPC * HD], BF16, kind="ExternalInput").ap()
    wo = nc.dram_tensor("wo", [HPC * HD, H], BF16, kind="ExternalInput").ap()
    biasv = nc.dram_tensor("biasv", [P, HPC], F32, kind="ExternalInput").ap()
    out2 = nc.dram_tensor("out2", [S, H], BF16, kind="ExternalOutput").ap()

    with tile.TileContext(nc) as tc, contextlib.ExitStack() as ctx:
        const = ctx.enter_context(tc.tile_pool(name="const", bufs=1))
        wvp = ctx.enter_context(tc.tile_pool(name="wvp", bufs=3))
        xkp = ctx.enter_context(tc.tile_pool(name="xkp", bufs=3))
        mpp = ctx.enter_context(tc.tile_pool(name="mpp", bufs=3))
        wop = ctx.enter_context(tc.tile_pool(name="wop", bufs=2))
        vres = ctx.enter_context(tc.tile_pool(name="vres", bufs=1))
        ores = ctx.enter_context(tc.tile_pool(name="ores", bufs=1))
        apool = ctx.enter_context(tc.tile_pool(name="apool", bufs=3))

        qt_sb = [const.tile([P, S], F16, name=f"qt{h}", tag=f"qt{h}")
                 for h in range(HPC)]
        ktk_sb = [const.tile([P, NKEPT], F16, name=f"ktk{h}", tag=f"ktk{h}")
                  for h in range(HPC)]
        wo_sb = [const.tile([P, H], BF16, name=f"wo{kk}", tag=f"wo{kk}")
                 for kk in range(HPC)]
        bias_sb = const.tile([P, HPC], F32, name="biasvt", tag="biasvt")
        keptq_sb = const.tile([P, HPC * 2], F32, name="keptq", tag="keptq")
        iota_i = const.tile([P, S], mybir.dt.int32, name="iotai", tag="iotai")
        nc.gpsimd.iota(iota_i[:], pattern=[[1, S]], base=0, channel_multiplier=0)
        iota_f = const.tile([P, S], F32, name="iotaf", tag="iotaf")
        nc.vector.tensor_copy(iota_f[:], iota_i[:])

        # v projection of kept rows: v_sb[h][t] = [128 kept, 128 d].
        # The big qt/ktk const loads are queued behind the first few v-proj
        # chunk DMAs so the PE starts within a few us.
        v_sb = [[vres.tile([P if t == 0 else T1, HD], BF16,
                           name=f"vsb{h}_{t}", tag=f"vsb{h}_{t}")
                 for t in range(2)] for h in range(HPC)]
        with tc.tile_pool(name="vps", bufs=1, space="PSUM") as vps:
            v_ps = [[vps.tile([P if t == 0 else T1, HD], F32,
                              name=f"vps{h}_{t}", tag=f"vps{h}_{t}")
                     for t in range(2)] for h in range(HPC)]
            for kc in range(KC):
                ksl = slice(kc * P, (kc + 1) * P)
                wvt = wvp.tile([P, HPC * HD], BF16, name="wvt", tag="wvt")
                nc.scalar.dma_start(wvt[:], wv[ksl, :])
                xkt = xkp.tile([P, HPC * NKEPT], BF16, name="xkt", tag="xkt")
                nc.sync.dma_start(xkt[:], xtk[ksl, :])
                if kc == 2:
                    for h in range(HPC):
                        nc.gpsimd.dma_start(ktk_sb[h][:],
                                            ktk[h * HD:(h + 1) * HD, :])
                    nc.gpsimd.dma_start(bias_sb[:], biasv[:, :])
                    nc.gpsimd.dma_start(keptq_sb[:], keptq[:, :])
                if kc == 4:
                    for h in range(HPC):
                        nc.gpsimd.dma_start(qt_sb[h][:],
                                            qtd[h * HD:(h + 1) * HD, :])
                if kc == 20:
                    for kk in range(HPC):
                        nc.scalar.dma_start(wo_sb[kk][:],
                                            wo[kk * P:(kk + 1) * P, :])
                for h in range(HPC):
                    for t in range(2):
                        nc.tensor.matmul(
                            v_ps[h][t][:],
                            lhsT=xkt[:, h * NKEPT + t * P: h * NKEPT +
                                     (P if t == 0 else NKEPT)],
                            rhs=wvt[:, h * HD:(h + 1) * HD],
                            start=(kc == 0), stop=(kc == KC - 1))
            for h in range(HPC):
                for t in range(2):
                    nc.vector.tensor_copy(v_sb[h][t][:], v_ps[h][t][:])

        # step masks (+1e9 where kept_idx <= q) precomputed per (h, t)
        mp_sb = [[mpp.tile([P, S], F32, name=f"mp{h}_{t}", tag=f"mp{h}_{t}",
                           bufs=1)
                  for t in range(2)] for h in range(HPC)]
        for h in range(HPC):
            for t in range(2):
                nc.vector.tensor_scalar(
                    mp_sb[h][t][:], iota_f[:],
                    keptq_sb[:, h * 2 + t: h * 2 + t + 1],
                    1e9, mybir.AluOpType.is_ge, mybir.AluOpType.mult)

        # per-head kept attention -> ohT bf16 [128 d, 2048 q], pipelined in
        # 1024-wide q blocks; pa(t=1) matmuls overlap the t=0 A-add (gpsimd)
        oh_sb = [ores.tile([P, S], BF16, name=f"oh{h}", tag=f"oh{h}")
                 for h in range(HPC)]
        with tc.tile_pool(name="pop", bufs=2, space="PSUM") as pop, \
             tc.tile_pool(name="pap", bufs=2, space="PSUM") as pap:
            for h in range(HPC):
                for half in range(2):
                    hsl = slice(half * 1024, (half + 1) * 1024)
                    po = pop.tile([P, 1024], F32, name="po", tag="po")
                    pa_t, A_t = [], []
                    for t in range(2):
                        rows = P if t == 0 else T1
                        pa = pap.tile([P, 1024], F32, name="pa", tag="pa")
                        for j in range(2):
                            qsl = slice(half * 1024 + j * 512,
                                        half * 1024 + (j + 1) * 512)
                            nc.tensor.matmul(
                                pa[0:rows, j * 512:(j + 1) * 512],
                                lhsT=ktk_sb[h][:, t * P: P if t == 0 else NKEPT],
                                rhs=qt_sb[h][:, qsl], start=True, stop=True)
                        A = apool.tile([P, 1024], BF16, name="A", tag="A")
                        nc.vector.tensor_add(A[0:rows, :], pa[0:rows, :],
                                             mp_sb[h][t][0:rows, hsl])
                        pa_t.append(pa)
                        A_t.append(A)
                    for t in range(2):
                        rows = P if t == 0 else T1
                        for j in range(2):
                            nc.tensor.matmul(
                                po[:, j * 512:(j + 1) * 512],
                                lhsT=v_sb[h][t][:],
                                rhs=A_t[t][0:rows, j * 512:(j + 1) * 512],
                                start=(t == 0), stop=(t == 1))
                    nc.vector.tensor_scalar_add(oh_sb[h][:, hsl], po[:],
                                                bias_sb[:, h:h + 1])

        # row-parallel o_proj: out2[s, :] partial (bf16); wo fully resident,
        # one wide out2 DMA per q-tile
        wps = ctx.enter_context(tc.tile_pool(name="wps", bufs=4, space="PSUM"))
        owp = ctx.enter_context(tc.tile_pool(name="owp", bufs=2))
        for qt in range(16):
            qsl = slice(qt * P, (qt + 1) * P)
            ow = owp.tile([P, H], BF16, name="ow", tag="ow")
            for nt in range(8):
                nsl = slice(nt * 512, (nt + 1) * 512)
                pw = wps.tile([P, 512], F32, name="pw", tag="pw")
                for kc in range(HPC):
                    nc.tensor.matmul(pw[:], lhsT=oh_sb[kc][:, qsl],
                                     rhs=wo_sb[kc][:, nsl],
                                     start=(kc == 0), stop=(kc == HPC - 1))
                nc.vector.tensor_copy(ow[:, nsl], pw[:])
            nc.sync.dma_start(out2[qsl, :], ow[:, :])
    nc.compile()
    return nc


def _topk_mask_indices(scores):
    """jax.lax.top_k semantics: descending, ties -> lower index."""
    s = scores[:-2]
    idx = np.argsort(-s, kind="stable")[:KEEP]
    kept = np.concatenate([idx, [S - 2, S - 1]])
    kept.sort()
    return kept.astype(np.int64)


def _is_causal(am):
    if am.shape != (S, S):
        return False
    k = np.arange(S)
    row0 = np.where(k <= 0, 0.0, -1e9).astype(np.float32)
    if not np.array_equal(am[0], row0):
        return False
    tri = np.where(k[:, None] >= k[None, :], 0.0, -1e9).astype(np.float32)
    return np.array_equal(am, tri)


def kernel(hidden_states, attention_mask, Wq, Wk, Wv, Wo, position_ids):
    am = np.ascontiguousarray(np.asarray(attention_mask, np.float32)[0, 0])
    if not _is_causal(am):
        return _kernel_general(hidden_states, am, Wq, Wk, Wv, Wo, position_ids)

    x = np.ascontiguousarray(np.asarray(hidden_states, np.float32)[0])   # [S, H]
    Wq = np.asarray(Wq, np.float32)
    Wk = np.asarray(Wk, np.float32)
    Wv = np.asarray(Wv, np.float32)
    Wo = np.asarray(Wo, np.float32)
    pos = np.asarray(position_ids)[0]

    inv = 1.0 / (10000.0 ** (np.arange(0, HD, 2, dtype=np.float32) / HD))
    fr = pos.astype(np.float32)[:, None] * inv
    emb = np.concatenate([fr, fr], -1)
    cosT = np.ascontiguousarray(np.cos(emb).astype(np.float16).T)  # [128, S]
    sinT = np.ascontiguousarray(np.sin(emb).astype(np.float16).T)
    xT = np.ascontiguousarray(x.T)                                  # [H, S]
    xTb = xT.astype(NPBF)
    identv = np.eye(P, dtype=np.float16)
    jj = np.arange(512)[None, :]
    pp = np.arange(P)[:, None]
    mtailv = np.concatenate(
        [np.where(jj > m * P + pp, np.float16(-60000), np.float16(0))
         for m in range(4)], axis=1)                                # [128, 2048]
    scale = np.float32(1.0 / np.sqrt(HD))

    if "p1" not in _cache:
        _cache["p1"] = _build_phase1()
    nc1 = _cache["p1"]

    in_maps = []
    for c in range(NCORES):
        hsl = slice(c * HPC * HD, (c + 1) * HPC * HD)
        in_maps.append({
            "xb": xTb,
            "wq": np.ascontiguousarray((Wq[hsl, :].T * scale).astype(NPBF)),
            "wk": np.ascontiguousarray(Wk[hsl, :].T.astype(NPBF)),
            "cos": cosT, "sin": sinT, "ident": identv, "mtail": mtailv,
        })
    r1 = _run(nc1, in_maps, list(range(NCORES)))
    _cache["exec1"] = r1.exec_time_ns

    # host: top-k + gathers
    xsum = x.astype(np.float64).sum(0)                               # [H]
    in_maps2 = []
    for c in range(NCORES):
        res = r1.results[c]
        scores = res["scores"]
        qt, kt = res["qt"], res["kt"]
        hsl = slice(c * HPC * HD, (c + 1) * HPC * HD)
        Wv_c = Wv[hsl, :]
        ktkv = np.zeros((HPC * HD, NKEPT), np.float16)
        keptqv = np.full((P, HPC * 2), 1e9, np.float32)
        xtkv = np.zeros((H, HPC * NKEPT), NPBF)
        for h in range(HPC):
            kept = _topk_mask_indices(scores[h])
            ktkv[h * HD:(h + 1) * HD, :] = kt[h * HD:(h + 1) * HD, kept]
            kf = kept.astype(np.float32)
            keptqv[:T1, h * 2 + 1] = kf[P:]
            keptqv[:, h * 2] = kf[:P]
            xtkv[:, h * NKEPT:(h + 1) * NKEPT] = x[kept, :].T.astype(NPBF)
        vsum = (xsum @ Wv_c.astype(np.float64).T)                    # [512]
        bias = (-1e9 * vsum).astype(np.float32).reshape(HPC, HD).T   # [128, 4]
        in_maps2.append({
            "qt2": qt, "ktk": ktkv, "keptq": keptqv, "xtk": xtkv,
            "wv": np.ascontiguousarray(Wv_c.T.astype(NPBF)),
            "wo": np.ascontiguousarray(Wo[:, hsl].T.astype(NPBF)),
            "biasv": np.ascontiguousarray(bias),
        })

    if "p2" not in _cache:
        _cache["p2"] = _build_phase2()
    nc2 = _cache["p2"]
    r2 = _run(nc2, in_maps2, list(range(NCORES)))
    _cache["exec2"] = r2.exec_time_ns

    out = np.zeros((S, H), np.float32)
    for c in range(NCORES):
        out += r2.results[c]["out2"].astype(np.float32)
    return out.reshape(1, S, H)


# ───────────────────────── general-mask fallback (fp32) ─────────────────────


def _build_phase1_general():
    nc = bacc.Bacc("TRN2", target_bir_lowering=False, debug=False,
                   num_devices=NCORES)
    xt = nc.dram_tensor("xt", [H, S], F32, kind="ExternalInput").ap()
    wq = nc.dram_tensor("wq", [H, HPC * HD], F32, kind="ExternalInput").ap()
    wk = nc.dram_tensor("wk", [H, HPC * HD], F32, kind="ExternalInput").ap()
    cosd = nc.dram_tensor("cos", [P, S], F32, kind="ExternalInput").ap()
    sind = nc.dram_tensor("sin", [P, S], F32, kind="ExternalInput").ap()
    maskd = nc.dram_tensor("mask", [S, S], F32, kind="ExternalInput").ap()
    scores_o = nc.dram_tensor("scores", [HPC, S], F32, kind="ExternalOutput").ap()
    qt_o = nc.dram_tensor("qt", [HPC * HD, S], F32, kind="ExternalOutput").ap()
    kt_o = nc.dram_tensor("kt", [HPC * HD, S], F32, kind="ExternalOutput").ap()

    with tile.TileContext(nc) as tc, contextlib.ExitStack() as ctx:
        const = ctx.enter_context(tc.tile_pool(name="const", bufs=1))
        wpool = ctx.enter_context(tc.tile_pool(name="wpool", bufs=1))
        xpool = ctx.enter_context(tc.tile_pool(name="xpool", bufs=4))
        qkres = ctx.enter_context(tc.tile_pool(name="qkres", bufs=1))
        rpool = ctx.enter_context(tc.tile_pool(name="rpool", bufs=2))
        mpool = ctx.enter_context(tc.tile_pool(name="mpool", bufs=3))
        epool = ctx.enter_context(tc.tile_pool(name="epool", bufs=3))
        vpool = ctx.enter_context(tc.tile_pool(name="vpool", bufs=2))

        cos_sb = const.tile([P, S], F32, name="cos", tag="cos")
        sin_sb = const.tile([P, S], F32, name="sin", tag="sin")
        nc.sync.dma_start(cos_sb[:], cosd[:, :])
        nc.sync.dma_start(sin_sb[:], sind[:, :])

        qt_sb = [qkres.tile([P, S], F32, name=f"qt{h}", tag=f"qt{h}")
                 for h in range(HPC)]
        kt_sb = [qkres.tile([P, S], F32, name=f"kt{h}", tag=f"kt{h}")
                 for h in range(HPC)]

        def proj_pass(ppool, wd, out_tiles, out_dram):
            w_sb = []
            for kc in range(KC):
                t = wpool.tile([P, HPC * HD], F32, name=f"w{kc}", tag=f"w{kc}")
                nc.sync.dma_start(t[:], wd[kc * P:(kc + 1) * P, :])
                w_sb.append(t)
            for sq in range(4):
                ssl = slice(sq * 512, (sq + 1) * 512)
                ps = [ppool.tile([P, 512], F32, name=f"pj{h}", tag=f"pj{h}")
                      for h in range(HPC)]
                for kc in range(KC):
                    xc = xpool.tile([P, 512], F32, name="xc", tag="xc")
                    nc.sync.dma_start(xc[:], xt[kc * P:(kc + 1) * P, ssl])
                    for h in range(HPC):
                        nc.tensor.matmul(
                            ps[h][:], lhsT=w_sb[kc][:, h * HD:(h + 1) * HD],
                            rhs=xc[:], start=(kc == 0), stop=(kc == KC - 1))
                for h in range(HPC):
                    dst = out_tiles[h][:, ssl]
                    m = rpool.tile([P, 512], F32, name="ropetmp", tag="ropetmp")
                    nc.vector.tensor_mul(m[:], ps[h][:], cos_sb[:, ssl])
                    rot = rpool.tile([P, 512], F32, name="roperot", tag="roperot")
                    nc.vector.tensor_scalar_mul(rot[0:64, :], ps[h][64:128, :], -1.0)
                    nc.vector.tensor_scalar_mul(rot[64:128, :], ps[h][0:64, :], 1.0)
                    rs_ = rpool.tile([P, 512], F32, name="ropesin", tag="ropesin")
                    nc.vector.tensor_mul(rs_[:], rot[:], sin_sb[:, ssl])
                    nc.vector.tensor_add(dst[:], m[:], rs_[:])
                    nc.sync.dma_start(out_dram[h * HD:(h + 1) * HD, ssl], dst)

        with tc.tile_pool(name="ppool", bufs=2, space="PSUM") as ppool:
            proj_pass(ppool, wq, qt_sb, qt_o)
            proj_pass(ppool, wk, kt_sb, kt_o)

        spool = ctx.enter_context(tc.tile_pool(name="spool", bufs=1, space="PSUM"))
        apool = ctx.enter_context(tc.tile_pool(name="apool", bufs=2, space="PSUM"))

        for h in range(HPC):
            sc_ps = spool.tile([1, S], F32, name="scps", tag="scps")
            for qt in range(16):
                qsl = slice(qt * P, (qt + 1) * P)
                E_half, rs_half = [], []
                for half in range(2):
                    hs = slice(half * 1024, (half + 1) * 1024)
                    aw = apool.tile([P, 1024], F32, name="aw", tag="aw")
                    for j in range(2):
                        nsl = slice(half * 1024 + j * 512,
                                    half * 1024 + (j + 1) * 512)
                        nc.tensor.matmul(
                            aw[:, j * 512:(j + 1) * 512],
                            lhsT=qt_sb[h][:, qsl], rhs=kt_sb[h][:, nsl],
                            start=True, stop=True)
                    mt = mpool.tile([P, 1024], F32, name="mt", tag="mt")
                    nc.sync.dma_start(mt[:], maskd[qsl, hs])
                    nc.vector.tensor_add(aw[:], aw[:], mt[:])
                    E = epool.tile([P, 1024], F32, name="E", tag="E")
                    rs = vpool.tile([P, 1], F32, name=f"rs{half}", tag=f"rs{half}")
                    nc.scalar.activation(E[:], aw[:], EXP, accum_out=rs[:])
                    E_half.append(E)
                    rs_half.append(rs)
                rtot = vpool.tile([P, 1], F32, name="rtot", tag="rtot")
                nc.vector.tensor_add(rtot[:], rs_half[0][:], rs_half[1][:])
                r = vpool.tile([P, 1], F32, name="r", tag="r")
                nc.vector.reciprocal(r[:], rtot[:])
                for hh in range(2):
                    for j in range(2):
                        osl = slice(hh * 1024 + j * 512, hh * 1024 + (j + 1) * 512)
                        nc.tensor.matmul(
                            sc_ps[:, osl], lhsT=r[:],
                            rhs=E_half[hh][:, j * 512:(j + 1) * 512],
                            start=(qt == 0), stop=(qt == 15))
            scsb = vpool.tile([1, S], F32, name="scsb", tag="scsb", bufs=1)
            nc.vector.tensor_copy(scsb[:], sc_ps[:])
            nc.sync.dma_start(scores_o[h:h + 1, :], scsb[:])
    nc.compile()
    return nc


def _build_phase2_general():
    nc = bacc.Bacc("TRN2", target_bir_lowering=False, debug=False,
                   num_devices=NCORES)
    qtd = nc.dram_tensor("qt2", [HPC * HD, S], F32, kind="ExternalInput").ap()
    ktk = nc.dram_tensor("ktk", [HPC * HD, KPAD], F32, kind="ExternalInput").ap()
    mpk = nc.dram_tensor("mpk", [HPC * KPAD, S], F32, kind="ExternalInput").ap()
    xtk = nc.dram_tensor("xtk", [H, HPC * KPAD], F32, kind="ExternalInput").ap()
    wv = nc.dram_tensor("wv", [H, HPC * HD], F32, kind="ExternalInput").ap()
    wo = nc.dram_tensor("wo", [HPC * HD, H], F32, kind="ExternalInput").ap()
    biasv = nc.dram_tensor("biasv", [P, HPC], F32, kind="ExternalInput").ap()
    out2 = nc.dram_tensor("out2", [S, H], F32, kind="ExternalOutput").ap()

    with tile.TileContext(nc) as tc, contextlib.ExitStack() as ctx:
        const = ctx.enter_context(tc.tile_pool(name="const", bufs=1))
        wvp = ctx.enter_context(tc.tile_pool(name="wvp", bufs=3))
        xkp = ctx.enter_context(tc.tile_pool(name="xkp", bufs=3))
        mpp = ctx.enter_context(tc.tile_pool(name="mpp", bufs=3))
        wop = ctx.enter_context(tc.tile_pool(name="wop", bufs=1))
        vres = ctx.enter_context(tc.tile_pool(name="vres", bufs=1))
        ores = ctx.enter_context(tc.tile_pool(name="ores", bufs=1))
        apool = ctx.enter_context(tc.tile_pool(name="apool", bufs=2))

        qt_sb = [const.tile([P, S], F32, name=f"qt{h}", tag=f"qt{h}")
                 for h in range(HPC)]
        for h in range(HPC):
            nc.sync.dma_start(qt_sb[h][:], qtd[h * HD:(h + 1) * HD, :])
        ktk_sb = [const.tile([P, KPAD], F32, name=f"ktk{h}", tag=f"ktk{h}")
                  for h in range(HPC)]
        for h in range(HPC):
            nc.sync.dma_start(ktk_sb[h][:], ktk[h * HD:(h + 1) * HD, :])
        bias_sb = const.tile([P, HPC], F32, name="biasvt", tag="biasvt")
        nc.sync.dma_start(bias_sb[:], biasv[:, :])

        v_sb = [[vres.tile([P, HD], F32, name=f"vsb{h}_{t}", tag=f"vsb{h}_{t}")
                 for t in range(2)] for h in range(HPC)]
        with tc.tile_pool(name="vps", bufs=1, space="PSUM") as vps:
            v_ps = [[vps.tile([P, HD], F32, name=f"vps{h}_{t}", tag=f"vps{h}_{t}")
                     for t in range(2)] for h in range(HPC)]
            for kc in range(KC):
                ksl = slice(kc * P, (kc + 1) * P)
                wvt = wvp.tile([P, HPC * HD], F32, name="wvt", tag="wvt")
                nc.sync.dma_start(wvt[:], wv[ksl, :])
                xkt = xkp.tile([P, HPC * KPAD], F32, name="xkt", tag="xkt")
                nc.sync.dma_start(xkt[:], xtk[ksl, :])
                for h in range(HPC):
                    for t in range(2):
                        nc.tensor.matmul(
                            v_ps[h][t][:],
                            lhsT=xkt[:, h * KPAD + t * P: h * KPAD + (t + 1) * P],
                            rhs=wvt[:, h * HD:(h + 1) * HD],
                            start=(kc == 0), stop=(kc == KC - 1))
            for h in range(HPC):
                for t in range(2):
                    nc.vector.tensor_copy(v_sb[h][t][:], v_ps[h][t][:])

        oh_sb = [ores.tile([P, S], F32, name=f"oh{h}", tag=f"oh{h}")
                 for h in range(HPC)]
        with tc.tile_pool(name="atp", bufs=1, space="PSUM") as atp:
            for h in range(HPC):
                po = atp.tile([P, S], F32, name="po", tag="po")
                for t in range(2):
                    pa = atp.tile([P, S], F32, name="pa", tag="pa")
                    for j in range(4):
                        qsl = slice(j * 512, (j + 1) * 512)
                        nc.tensor.matmul(
                            pa[:, qsl],
                            lhsT=ktk_sb[h][:, t * P:(t + 1) * P],
                            rhs=qt_sb[h][:, qsl], start=True, stop=True)
                    mp = mpp.tile([P, S], F32, name="mp", tag="mp")
                    nc.sync.dma_start(
                        mp[:], mpk[h * KPAD + t * P: h * KPAD + (t + 1) * P, :])
                    A = apool.tile([P, S], F32, name="A", tag="A")
                    nc.vector.tensor_add(A[:], pa[:], mp[:])
                    for j in range(4):
                        qsl = slice(j * 512, (j + 1) * 512)
                        nc.tensor.matmul(
                            po[:, qsl], lhsT=v_sb[h][t][:], rhs=A[:, qsl],
                            start=(t == 0), stop=(t == 1))
                nc.vector.tensor_scalar_add(oh_sb[h][:], po[:],
                                            bias_sb[:, h:h + 1])

        wps = ctx.enter_context(tc.tile_pool(name="wps", bufs=4, space="PSUM"))
        for nt in range(8):
            nsl = slice(nt * 512, (nt + 1) * 512)
            wot = [wop.tile([P, 512], F32, name=f"wot{kc}", tag=f"wot{kc}")
                   for kc in range(HPC)]
            for kc in range(HPC):
                nc.sync.dma_start(wot[kc][:], wo[kc * P:(kc + 1) * P, nsl])
            for qt in range(16):
                qsl = slice(qt * P, (qt + 1) * P)
                pw = wps.tile([P, 512], F32, name="pw", tag="pw")
                for kc in range(HPC):
                    nc.tensor.matmul(pw[:], lhsT=oh_sb[kc][:, qsl],
                                     rhs=wot[kc][:],
                                     start=(kc == 0), stop=(kc == HPC - 1))
                ow = apool.tile([P, 512], F32, name="ow", tag="ow")
                nc.vector.tensor_copy(ow[:], pw[:])
                nc.sync.dma_start(out2[qsl, nsl], ow[:])
    nc.compile()
    return nc


def _kernel_general(hidden_states, am, Wq, Wk, Wv, Wo, position_ids):
    x = np.ascontiguousarray(np.asarray(hidden_states, np.float32)[0])   # [S, H]
    Wq = np.asarray(Wq, np.float32)
    Wk = np.asarray(Wk, np.float32)
    Wv = np.asarray(Wv, np.float32)
    Wo = np.asarray(Wo, np.float32)
    pos = np.asarray(position_ids)[0]

    inv = 1.0 / (10000.0 ** (np.arange(0, HD, 2, dtype=np.float32) / HD))
    fr = pos.astype(np.float32)[:, None] * inv
    emb = np.concatenate([fr, fr], -1)
    cosT = np.ascontiguousarray(np.cos(emb).astype(np.float32).T)  # [128, S]
    sinT = np.ascontiguousarray(np.sin(emb).astype(np.float32).T)
    xT = np.ascontiguousarray(x.T)                                  # [H, S]
    scale = np.float32(1.0 / np.sqrt(HD))

    if "g1" not in _cache:
        _cache["g1"] = _build_phase1_general()
    nc1 = _cache["g1"]

    in_maps = []
    for c in range(NCORES):
        hsl = slice(c * HPC * HD, (c + 1) * HPC * HD)
        in_maps.append({
            "xt": xT,
            "wq": np.ascontiguousarray(Wq[hsl, :].T * scale),
            "wk": np.ascontiguousarray(Wk[hsl, :].T),
            "cos": cosT, "sin": sinT, "mask": am,
        })
    r1 = _run(nc1, in_maps, list(range(NCORES)))
    _cache["exec1"] = r1.exec_time_ns

    xsum = x.astype(np.float64).sum(0)                               # [H]
    in_maps2 = []
    for c in range(NCORES):
        res = r1.results[c]
        scores, qt, kt = res["scores"], res["qt"], res["kt"]
        hsl = slice(c * HPC * HD, (c + 1) * HPC * HD)
        Wv_c = Wv[hsl, :]
        ktkv = np.zeros((HPC * HD, KPAD), np.float32)
        mpkv = np.zeros((HPC * KPAD, S), np.float32)
        xtkv = np.zeros((H, HPC * KPAD), np.float32)
        for h in range(HPC):
            kept = _topk_mask_indices(scores[h])
            ktkv[h * HD:(h + 1) * HD, :NKEPT] = kt[h * HD:(h + 1) * HD, kept]
            mpkv[h * KPAD: h * KPAD + NKEPT, :] = am[:, kept].T + np.float32(1e9)
            xtkv[:, h * KPAD: h * KPAD + NKEPT] = x[kept, :].T
        vsum = (xsum @ Wv_c.astype(np.float64).T)                    # [512]
        bias = (-1e9 * vsum).astype(np.float32).reshape(HPC, HD).T   # [128, 4]
        in_maps2.append({
            "qt2": qt, "ktk": ktkv, "mpk": mpkv, "xtk": xtkv,
            "wv": np.ascontiguousarray(Wv_c.T),
            "wo": np.ascontiguousarray(Wo[:, hsl].T),
            "biasv": np.ascontiguousarray(bias),
        })

    if "g2" not in _cache:
        _cache["g2"] = _build_phase2_general()
    nc2 = _cache["g2"]
    r2 = _run(nc2, in_maps2, list(range(NCORES)))
    _cache["exec2"] = r2.exec_time_ns

    out = np.zeros((S, H), np.float32)
    for c in range(NCORES):
        out += r2.results[c]["out2"]
    return out.reshape(1, S, H)


# revision 58
# speedup vs baseline: 1.0519x; 1.0519x over previous
"""Trainium2 Bass kernel for nn_LlamaAttention_kvcache (sparse H2O attention).

Strategy (8 NeuronCores, tensor-parallel over heads, 4 heads/core):
  Phase 1 (device): q/k projections (scale folded into Wq) in bf16, RoPE,
    causal-only QK^T (lower-triangular k-slices per q-tile), exp via the
    activation engine (row sums piggybacked on accum_out; diagonal slice
    masked post-exp with gpsimd affine_select), per-head softmax column
    scores accumulated with rank-1 (1/rowsum)^T @ E matmuls.
  Host: exact top-k selection per head (matches jax.lax.top_k tie-breaking),
    gathers kept k-columns / x-rows.
  Phase 2 (device): kept-column scores + eviction decomposition
      aw_new @ v = [M o (aw + 1e9)] @ v_kept  -  1e9 * (sum_all v)
    (evicted columns contribute exactly -1e9 * v); the +1e9*step mask is
    generated on-device from kept indices (iota + tensor_scalar is_ge);
    then row-parallel o_proj in bf16. Host sums the 8 per-core partials.
  Falls back to a general (mask-as-data, fp32) path when attention_mask is
  not the standard causal mask.
"""

import contextlib
import sys

for p in ("/opt/trn_rl_repo", "/root/.axon_site/_ro/trn_rl_repo"):
    if p not in sys.path:
        sys.path.append(p)

import ml_dtypes
import numpy as np

import concourse.bacc as bacc
import concourse.mybir as mybir
import concourse.tile as tile
from concourse.bass_utils import run_bass_kernel_spmd

F32 = mybir.dt.float32
F32R = mybir.dt.float32r
F16 = mybir.dt.float16
BF16 = mybir.dt.bfloat16
NPBF = ml_dtypes.bfloat16
P = 128
S = 2048
H = 4096
NH = 32
HD = 128
NCORES = 8
HPC = NH // NCORES          # heads per core = 4
KC = H // P                 # 32 k-chunks over the 4096 contraction
KEEP = int(0.1 * S)         # 204 top-k heavy hitters
NKEPT = KEEP + 2            # + last-2 local tokens = 206
T1 = NKEPT - P              # ragged second kept-tile rows = 78
KPAD = 256                  # padded kept count (general fallback only)
EXP = mybir.ActivationFunctionType.Exp

_cache = {}


def _run(nc, in_maps, core_ids):
    return run_bass_kernel_spmd(nc, in_maps, core_ids)


def _build_phase1():
    nc = bacc.Bacc("TRN2", target_bir_lowering=False, debug=False,
                   num_devices=NCORES)
    xb = nc.dram_tensor("xb", [H, S], BF16, kind="ExternalInput").ap()
    wq = nc.dram_tensor("wq", [H, HPC * HD], BF16, kind="ExternalInput").ap()
    wk = nc.dram_tensor("wk", [H, HPC * HD], BF16, kind="ExternalInput").ap()
    cosd = nc.dram_tensor("cos", [P, S], F16, kind="ExternalInput").ap()
    sind = nc.dram_tensor("sin", [P, S], F16, kind="ExternalInput").ap()
    identd = nc.dram_tensor("ident", [P, P], F16, kind="ExternalInput").ap()
    mtaild = nc.dram_tensor("mtail", [P, 4 * 512], F16, kind="ExternalInput").ap()
    scores_o = nc.dram_tensor("scores", [HPC, S], F32,
                              kind="ExternalOutput").ap()
    qt_o = nc.dram_tensor("qt", [HPC * HD, S], F16, kind="ExternalOutput").ap()
    kt_o = nc.dram_tensor("kt", [HPC * HD, S], F16, kind="ExternalOutput").ap()

    with tile.TileContext(nc) as tc, contextlib.ExitStack() as ctx:
        const = ctx.enter_context(tc.tile_pool(name="const", bufs=1))
        wpool = ctx.enter_context(tc.tile_pool(name="wpool", bufs=1))
        xpool = ctx.enter_context(tc.tile_pool(name="xpool", bufs=10))
        qkres = ctx.enter_context(tc.tile_pool(name="qkres", bufs=1))
        rpool = ctx.enter_context(tc.tile_pool(name="rpool", bufs=2))
        epool = ctx.enter_context(tc.tile_pool(name="epool", bufs=3))
        vpool = ctx.enter_context(tc.tile_pool(name="vpool", bufs=3))

        cos_sb = const.tile([P, S], F16, name="cos", tag="cos")
        sin_sb = const.tile([P, S], F16, name="sin", tag="sin")
        ident_sb = const.tile([P, P], F16, name="ident", tag="ident")
        mtail_sb = const.tile([P, 4 * 512], F16, name="mtail", tag="mtail")
        nc.gpsimd.dma_start(ident_sb[:], identd[:, :])
        nc.gpsimd.dma_start(mtail_sb[:], mtaild[:, :])

        # resident roped q/k per head, fp16: [128 d, 2048 s]
        qt_sb = [qkres.tile([P, S], F16, name=f"qt{h}", tag=f"qt{h}")
                 for h in range(HPC)]
        kt_sb = [qkres.tile([P, S], F16, name=f"kt{h}", tag=f"kt{h}")
                 for h in range(HPC)]

        # w tiles are DMA'd lazily inside the sq=0 loop (interleaved with x
        # chunks) so the first matmul isn't stuck behind an 8MB preamble
        wq_sb = [wpool.tile([P, HPC * HD], BF16, name=f"wq{kc}", tag=f"wq{kc}")
                 for kc in range(KC)]
        wk_sb = [wpool.tile([P, HPC * HD], BF16, name=f"wk{kc}", tag=f"wk{kc}")
                 for kc in range(KC)]

        def rope(ps, out_slice, ssl):
            m = rpool.tile([P, 512], F32, name="ropetmp", tag="ropetmp")
            nc.vector.tensor_mul(m[:], ps[:], cos_sb[:, ssl])
            rot = rpool.tile([P, 512], F32, name="roperot", tag="roperot")
            nc.vector.tensor_scalar_mul(rot[0:64, :], ps[64:128, :], -1.0)
            nc.vector.tensor_scalar_mul(rot[64:128, :], ps[0:64, :], 1.0)
            rs_ = rpool.tile([P, 512], F32, name="ropesin", tag="ropesin")
            nc.vector.tensor_mul(rs_[:], rot[:], sin_sb[:, ssl])
            nc.vector.tensor_add(out_slice, m[:], rs_[:])

        # two passes (q then k) per s-quarter with double-buffered PSUM so
        # the next pass's matmuls overlap this pass's RoPE drain; x chunks
        # are resident per quarter (one DMA, reused by both passes, with
        # per-tag rolling prefetch into the next quarter)
        with tc.tile_pool(name="ppool", bufs=2, space="PSUM") as ppool:
            for sq in range(4):          # 512-wide s quarters
                ssl = slice(sq * 512, (sq + 1) * 512)
                xc_sq = []
                for w_sb, wd, out_tiles, is_q in (
                        (wq_sb, wq, qt_sb, True), (wk_sb, wk, kt_sb, False)):
                    ps = [ppool.tile([P, 512], F32, name=f"pj{h}", tag=f"pj{h}")
                          for h in range(HPC)]
                    for kc in range(KC):
                        if is_q:
                            xc = xpool.tile([P, 512], BF16, name="xc",
                                            tag=f"xc{kc}", bufs=1)
                            nc.sync.dma_start(xc[:], xb[kc * P:(kc + 1) * P, ssl])
                            xc_sq.append(xc)
                        else:
                            xc = xc_sq[kc]
                        if sq == 0:
                            nc.scalar.dma_start(w_sb[kc][:],
                                                wd[kc * P:(kc + 1) * P, :])
                            if is_q and kc == 2:
                                nc.gpsimd.dma_start(cos_sb[:], cosd[:, :])
                                nc.gpsimd.dma_start(sin_sb[:], sind[:, :])
                        for h in range(HPC):
                            nc.tensor.matmul(
                                ps[h][:], lhsT=w_sb[kc][:, h * HD:(h + 1) * HD],
                                rhs=xc[:], start=(kc == 0), stop=(kc == KC - 1))
                    for h in range(HPC):
                        rope(ps[h], out_tiles[h][:, ssl], ssl)
        for h in range(HPC):
            nc.sync.dma_start(qt_o[h * HD:(h + 1) * HD, :], qt_sb[h][:, :])
            nc.sync.dma_start(kt_o[h * HD:(h + 1) * HD, :], kt_sb[h][:, :])

        spool = ctx.enter_context(tc.tile_pool(name="spool", bufs=1, space="PSUM"))
        apool = ctx.enter_context(tc.tile_pool(name="apool", bufs=2, space="PSUM"))

        # causal attention scores per head. The strictly-future entries of
        # the diagonal 512-slice get -60000 added in PSUM via an
        # identity-weight matmul (exp then yields exact zeros), so every
        # activation is uniform with accum_out and slices pair into
        # 1024-wide activations. Score matmuls write each 512-slice to its
        # own partition of a [4, 512] one-bank accumulator and lag 2
        # q-tiles so the PE never waits on the exp/rowsum chain.
        for h in range(HPC):
            sc_ps = spool.tile([1, S], F32, name="scps", tag="scps")
            pend = []

            def flush_one():
                oqt, oE, orb, onsl = pend.pop(0)
                for s in range(onsl):
                    ksl = slice(s * 512, (s + 1) * 512)
                    nc.tensor.matmul(sc_ps[:, ksl], lhsT=orb[:],
                                     rhs=oE[:, ksl],
                                     start=(oqt == 4 * s), stop=(oqt == 15))

            for qt in range(16):
                qsl = slice(qt * P, (qt + 1) * P)
                nsl = (qt + 1 + 3) // 4      # 512-slices overlapping [0,(qt+1)*128)
                m = qt % 4                   # within-slice diagonal offset
                E = epool.tile([P, S], BF16, name="E", tag="E")
                r_parts = []
                for pr in range((nsl + 1) // 2):
                    s0 = 2 * pr
                    w = 1024 if s0 + 1 < nsl else 512
                    aw = apool.tile([P, 1024], F32, name="aw", tag="aw")
                    for k in range(w // 512):
                        s = s0 + k
                        ksl = slice(s * 512, (s + 1) * 512)
                        awsl = aw[:, k * 512:(k + 1) * 512]
                        if s == nsl - 1:
                            nc.tensor.matmul(awsl, lhsT=qt_sb[h][:, qsl],
                                             rhs=kt_sb[h][:, ksl],
                                             start=True, stop=False)
                            nc.tensor.matmul(
                                awsl, lhsT=ident_sb[:],
                                rhs=mtail_sb[:, m * 512:(m + 1) * 512],
                                start=False, stop=True)
                        else:
                            nc.tensor.matmul(awsl, lhsT=qt_sb[h][:, qsl],
                                             rhs=kt_sb[h][:, ksl],
                                             start=True, stop=True)
                    rs = vpool.tile([P, 1], F32, name=f"rs{pr}", tag=f"rs{pr}")
                    nc.scalar.activation(E[:, s0 * 512: s0 * 512 + w],
                                         aw[:, 0:w], EXP, accum_out=rs[:])
                    r_parts.append(rs)
                racc = r_parts[0]
                if len(r_parts) > 1:
                    racc = vpool.tile([P, 1], F32, name="racc", tag="racc")
                    nc.vector.tensor_add(racc[:], r_parts[0][:], r_parts[1][:])
                rinv = vpool.tile([P, 1], F32, name="rinv", tag="rinv")
                nc.vector.reciprocal(rinv[:], racc[:])
                rb = vpool.tile([P, 1], BF16, name="rb", tag="rb")
                nc.vector.tensor_copy(rb[:], rinv[:])
                pend.append((qt, E, rb, nsl))
                if len(pend) > 2:
                    flush_one()
            while pend:
                flush_one()
            scsb = vpool.tile([1, S], F32, name="scsb", tag="scsb", bufs=2)
            nc.vector.tensor_copy(scsb[:], sc_ps[:])
            nc.sync.dma_start(scores_o[h:h + 1, :], scsb[:])
    nc.compile()
    return nc


def _build_phase2():
    nc = bacc.Bacc("TRN2", target_bir_lowering=False, debug=False,
                   num_devices=NCORES)
    qtd = nc.dram_tensor("qt2", [HPC * HD, S], F16, kind="ExternalInput").ap()
    ktk = nc.dram_tensor("ktk", [HPC * HD, NKEPT], F16, kind="ExternalInput").ap()
    keptq = nc.dram_tensor("keptq", [P, HPC * 2], F32, kind="ExternalInput").ap()
    xtk = nc.dram_tensor("xtk", [H, HPC * NKEPT], BF16, kind="ExternalInput").ap()
    wv = nc.dram_tensor("wv", [H, HPC * HD], BF16, kind="ExternalInput").ap()
    wo = nc.dram_tensor("wo", [HPC * HD, H], BF16, kind="ExternalInput").ap()
    biasv = nc.dram_tensor("biasv", [P, HPC], F32, kind="ExternalInput").ap()
    out2 = nc.dram_tensor("out2", [S, H], BF16, kind="ExternalOutput").ap()

    with tile.TileContext(nc) as tc, contextlib.ExitStack() as ctx:
        const = ctx.enter_context(tc.tile_pool(name="const", bufs=1))
        wvp = ctx.enter_context(tc.tile_pool(name="wvp", bufs=3))
        xkp = ctx.enter_context(tc.tile_pool(name="xkp", bufs=3))
        mpp = ctx.enter_context(tc.tile_pool(name="mpp", bufs=3))
        wop = ctx.enter_context(tc.tile_pool(name="wop", bufs=2))
        vres = ctx.enter_context(tc.tile_pool(name="vres", bufs=1))
        ores = ctx.enter_context(tc.tile_pool(name="ores", bufs=1))
        apool = ctx.enter_context(tc.tile_pool(name="apool", bufs=3))

        qt_sb = [const.tile([P, S], F16, name=f"qt{h}", tag=f"qt{h}")
                 for h in range(HPC)]
        ktk_sb = [const.tile([P, NKEPT], F16, name=f"ktk{h}", tag=f"ktk{h}")
                  for h in range(HPC)]
        wo_sb = [const.tile([P, H], BF16, name=f"wo{kk}", tag=f"wo{kk}")
                 for kk in range(HPC)]
        bias_sb = const.tile([P, HPC], F32, name="biasvt", tag="biasvt")
        keptq_sb = const.tile([P, HPC * 2], F32, name="keptq", tag="keptq")
        iota_i = const.tile([P, S], mybir.dt.int32, name="iotai", tag="iotai")
        nc.gpsimd.iota(iota_i[:], pattern=[[1, S]], base=0, channel_multiplier=0)
        iota_f = const.tile([P, S], F32, name="iotaf", tag="iotaf")
        nc.vector.tensor_copy(iota_f[:], iota_i[:])

        # v projection of kept rows: v_sb[h][t] = [128 kept, 128 d].
        # The big qt/ktk const loads are queued behind the first few v-proj
        # chunk DMAs so the PE starts within a few us.
        v_sb = [[vres.tile([P if t == 0 else T1, HD], BF16,
                           name=f"vsb{h}_{t}", tag=f"vsb{h}_{t}")
                 for t in range(2)] for h in range(HPC)]
        with tc.tile_pool(name="vps", bufs=1, space="PSUM") as vps:
            v_ps = [[vps.tile([P if t == 0 else T1, HD], F32,
                              name=f"vps{h}_{t}", tag=f"vps{h}_{t}")
                     for t in range(2)] for h in range(HPC)]
            for kc in range(KC):
                ksl = slice(kc * P, (kc + 1) * P)
                wvt = wvp.tile([P, HPC * HD], BF16, name="wvt", tag="wvt")
                nc.scalar.dma_start(wvt[:], wv[ksl, :])
                xkt = xkp.tile([P, HPC * NKEPT], BF16, name="xkt", tag="xkt")
                nc.sync.dma_start(xkt[:], xtk[ksl, :])
                if kc == 2:
                    for h in range(HPC):
                        nc.gpsimd.dma_start(ktk_sb[h][:],
                                            ktk[h * HD:(h + 1) * HD, :])
                    nc.gpsimd.dma_start(bias_sb[:], biasv[:, :])
                    nc.gpsimd.dma_start(keptq_sb[:], keptq[:, :])
                if kc == 4:
                    for h in range(HPC):
                        nc.gpsimd.dma_start(qt_sb[h][:],
                                            qtd[h * HD:(h + 1) * HD, :])
                if kc == 20:
                    for kk in range(HPC):
                        nc.scalar.dma_start(wo_sb[kk][:],
                                            wo[kk * P:(kk + 1) * P, :])
                for h in range(HPC):
                    for t in range(2):
                        nc.tensor.matmul(
                            v_ps[h][t][:],
                            lhsT=xkt[:, h * NKEPT + t * P: h * NKEPT +
                                     (P if t == 0 else NKEPT)],
                            rhs=wvt[:, h * HD:(h + 1) * HD],
                            start=(kc == 0), stop=(kc == KC - 1))
            for h in range(HPC):
                for t in range(2):
                    nc.vector.tensor_copy(v_sb[h][t][:], v_ps[h][t][:])

        # step masks (+1e9 where kept_idx <= q) precomputed per (h, t)
        mp_sb = [[mpp.tile([P, S], F32, name=f"mp{h}_{t}", tag=f"mp{h}_{t}",
                           bufs=1)
                  for t in range(2)] for h in range(HPC)]
        for h in range(HPC):
            for t in range(2):
                nc.vector.tensor_scalar(
                    mp_sb[h][t][:], iota_f[:],
                    keptq_sb[:, h * 2 + t: h * 2 + t + 1],
                    1e9, mybir.AluOpType.is_ge, mybir.AluOpType.mult)

        # per-head kept attention -> ohT bf16 [128 d, 2048 q], pipelined in
        # 1024-wide q blocks; pa(t=1) matmuls overlap the t=0 A-add (gpsimd)
        oh_sb = [ores.tile([P, S], BF16, name=f"oh{h}", tag=f"oh{h}")
                 for h in range(HPC)]
        with tc.tile_pool(name="pop", bufs=2, space="PSUM") as pop, \
             tc.tile_pool(name="pap", bufs=2, space="PSUM") as pap:
            for h in range(HPC):
                for half in range(2):
                    hsl = slice(half * 1024, (half + 1) * 1024)
                    po = pop.tile([P, 1024], F32, name="po", tag="po")
                    pa_t, A_t = [], []
                    for t in range(2):
                        rows = P if t == 0 else T1
                        pa = pap.tile([P, 1024], F32, name="pa", tag="pa")
                        for j in range(2):
                            qsl = slice(half * 1024 + j * 512,
                                        half * 1024 + (j + 1) * 512)
                            nc.tensor.matmul(
                                pa[0:rows, j * 512:(j + 1) * 512],
                                lhsT=ktk_sb[h][:, t * P: P if t == 0 else NKEPT],
                                rhs=qt_sb[h][:, qsl], start=True, stop=True)
                        A = apool.tile([P, 1024], BF16, name="A", tag="A")
                        nc.vector.tensor_add(A[0:rows, :], pa[0:rows, :],
                                             mp_sb[h][t][0:rows, hsl])
                        pa_t.append(pa)
                        A_t.append(A)
                    for t in range(2):
                        rows = P if t == 0 else T1
                        for j in range(2):
                            nc.tensor.matmul(
                                po[:, j * 512:(j + 1) * 512],
                                lhsT=v_sb[h][t][:],
                                rhs=A_t[t][0:rows, j * 512:(j + 1) * 512],
                                start=(t == 0), stop=(t == 1))
                    nc.vector.tensor_scalar_add(oh_sb[h][:, hsl], po[:],
                                                bias_sb[:, h:h + 1])

        # row-parallel o_proj: out2[s, :] partial (bf16); wo fully resident,
        # one wide out2 DMA per q-tile
        wps = ctx.enter_context(tc.tile_pool(name="wps", bufs=4, space="PSUM"))
        owp = ctx.enter_context(tc.tile_pool(name="owp", bufs=2))
        for qt in range(16):
            qsl = slice(qt * P, (qt + 1) * P)
            ow = owp.tile([P, H], BF16, name="ow", tag="ow")
            for nt in range(8):
                nsl = slice(nt * 512, (nt + 1) * 512)
                pw = wps.tile([P, 512], F32, name="pw", tag="pw")
                for kc in range(HPC):
                    nc.tensor.matmul(pw[:], lhsT=oh_sb[kc][:, qsl],
                                     rhs=wo_sb[kc][:, nsl],
                                     start=(kc == 0), stop=(kc == HPC - 1))
                nc.vector.tensor_copy(ow[:, nsl], pw[:])
            nc.sync.dma_start(out2[qsl, :], ow[:, :])
    nc.compile()
    return nc


def _topk_mask_indices(scores):
    """jax.lax.top_k semantics: descending, ties -> lower index."""
    s = scores[:-2]
    idx = np.argsort(-s, kind="stable")[:KEEP]
    kept = np.concatenate([idx, [S - 2, S - 1]])
    kept.sort()
    return kept.astype(np.int64)


def _is_causal(am):
    if am.shape != (S, S):
        return False
    k = np.arange(S)
    row0 = np.where(k <= 0, 0.0, -1e9).astype(np.float32)
    if not np.array_equal(am[0], row0):
        return False
    tri = np.where(k[:, None] >= k[None, :], 0.0, -1e9).astype(np.float32)
    return np.array_equal(am, tri)


def kernel(hidden_states, attention_mask, Wq, Wk, Wv, Wo, position_ids):
    am = np.ascontiguousarray(np.asarray(attention_mask, np.float32)[0, 0])
    if not _is_causal(am):
        return _kernel_general(hidden_states, am, Wq, Wk, Wv, Wo, position_ids)

    x = np.ascontiguousarray(np.asarray(hidden_states, np.float32)[0])   # [S, H]
    Wq = np.asarray(Wq, np.float32)
    Wk = np.asarray(Wk, np.float32)
    Wv = np.asarray(Wv, np.float32)
    Wo = np.asarray(Wo, np.float32)
    pos = np.asarray(position_ids)[0]

    inv = 1.0 / (10000.0 ** (np.arange(0, HD, 2, dtype=np.float32) / HD))
    fr = pos.astype(np.float32)[:, None] * inv
    emb = np.concatenate([fr, fr], -1)
    cosT = np.ascontiguousarray(np.cos(emb).astype(np.float16).T)  # [128, S]
    sinT = np.ascontiguousarray(np.sin(emb).astype(np.float16).T)
    xT = np.ascontiguousarray(x.T)                                  # [H, S]
    xTb = xT.astype(NPBF)
    identv = np.eye(P, dtype=np.float16)
    jj = np.arange(512)[None, :]
    pp = np.arange(P)[:, None]
    mtailv = np.concatenate(
        [np.where(jj > m * P + pp, np.float16(-60000), np.float16(0))
         for m in range(4)], axis=1)                                # [128, 2048]
    scale = np.float32(1.0 / np.sqrt(HD))

    if "p1" not in _cache:
        _cache["p1"] = _build_phase1()
    nc1 = _cache["p1"]

    in_maps = []
    for c in range(NCORES):
        hsl = slice(c * HPC * HD, (c + 1) * HPC * HD)
        in_maps.append({
            "xb": xTb,
            "wq": np.ascontiguousarray((Wq[hsl, :].T * scale).astype(NPBF)),
            "wk": np.ascontiguousarray(Wk[hsl, :].T.astype(NPBF)),
            "cos": cosT, "sin": sinT, "ident": identv, "mtail": mtailv,
        })
    r1 = _run(nc1, in_maps, list(range(NCORES)))
    _cache["exec1"] = r1.exec_time_ns

    # host: top-k + gathers
    xsum = x.astype(np.float64).sum(0)                               # [H]
    in_maps2 = []
    for c in range(NCORES):
        res = r1.results[c]
        scores = res["scores"]
        qt, kt = res["qt"], res["kt"]
        hsl = slice(c * HPC * HD, (c + 1) * HPC * HD)
        Wv_c = Wv[hsl, :]
        ktkv = np.zeros((HPC * HD, NKEPT), np.float16)
        keptqv = np.full((P, HPC * 2), 1e9, np.float32)
        xtkv = np.zeros((H, HPC * NKEPT), NPBF)
        for h in range(HPC):
            kept = _topk_mask_indices(scores[h])
            ktkv[h * HD:(h + 1) * HD, :] = kt[h * HD:(h + 1) * HD, kept]
            kf = kept.astype(np.float32)
            keptqv[:T1, h * 2 + 1] = kf[P:]
            keptqv[:, h * 2] = kf[:P]
            xtkv[:, h * NKEPT:(h + 1) * NKEPT] = x[kept, :].T.astype(NPBF)
        vsum = (xsum @ Wv_c.astype(np.float64).T)                    # [512]
        bias = (-1e9 * vsum).astype(np.float32).reshape(HPC, HD).T   # [128, 4]
        in_maps2.append({
            "qt2": qt, "ktk": ktkv, "keptq": keptqv, "xtk": xtkv,
            "wv": np.ascontiguousarray(Wv_c.T.astype(NPBF)),
            "wo": np.ascontiguousarray(Wo[:, hsl].T.astype(NPBF)),
            "biasv": np.ascontiguousarray(bias),
        })

    if "p2" not in _cache:
        _cache["p2"] = _build_phase2()
    nc2 = _cache["p2"]
    r2 = _run(nc2, in_maps2, list(range(NCORES)))
    _cache["exec2"] = r2.exec_time_ns

    out = np.zeros((S, H), np.float32)
    for c in range(NCORES):
        out += r2.results[c]["out2"].astype(np.float32)
    return out.reshape(1, S, H)


# ───────────────────────── general-mask fallback (fp32) ─────────────────────


def _build_phase1_general():
    nc = bacc.Bacc("TRN2", target_bir_lowering=False, debug=False,
                   num_devices=NCORES)
    xt = nc.dram_tensor("xt", [H, S], F32, kind="ExternalInput").ap()
    wq = nc.dram_tensor("wq", [H, HPC * HD], F32, kind="ExternalInput").ap()
    wk = nc.dram_tensor("wk", [H, HPC * HD], F32, kind="ExternalInput").ap()
    cosd = nc.dram_tensor("cos", [P, S], F32, kind="ExternalInput").ap()
    sind = nc.dram_tensor("sin", [P, S], F32, kind="ExternalInput").ap()
    maskd = nc.dram_tensor("mask", [S, S], F32, kind="ExternalInput").ap()
    scores_o = nc.dram_tensor("scores", [HPC, S], F32, kind="ExternalOutput").ap()
    qt_o = nc.dram_tensor("qt", [HPC * HD, S], F32, kind="ExternalOutput").ap()
    kt_o = nc.dram_tensor("kt", [HPC * HD, S], F32, kind="ExternalOutput").ap()

    with tile.TileContext(nc) as tc, contextlib.ExitStack() as ctx:
        const = ctx.enter_context(tc.tile_pool(name="const", bufs=1))
        wpool = ctx.enter_context(tc.tile_pool(name="wpool", bufs=1))
        xpool = ctx.enter_context(tc.tile_pool(name="xpool", bufs=4))
        qkres = ctx.enter_context(tc.tile_pool(name="qkres", bufs=1))
        rpool = ctx.enter_context(tc.tile_pool(name="rpool", bufs=2))
        mpool = ctx.enter_context(tc.tile_pool(name="mpool", bufs=3))
        epool = ctx.enter_context(tc.tile_pool(name="epool", bufs=3))
        vpool = ctx.enter_context(tc.tile_pool(name="vpool", bufs=2))

        cos_sb = const.tile([P, S], F32, name="cos", tag="cos")
        sin_sb = const.tile([P, S], F32, name="sin", tag="sin")
        nc.sync.dma_start(cos_sb[:], cosd[:, :])
        nc.sync.dma_start(sin_sb[:], sind[:, :])

        qt_sb = [qkres.tile([P, S], F32, name=f"qt{h}", tag=f"qt{h}")
                 for h in range(HPC)]
        kt_sb = [qkres.tile([P, S], F32, name=f"kt{h}", tag=f"kt{h}")
                 for h in range(HPC)]

        def proj_pass(ppool, wd, out_tiles, out_dram):
            w_sb = []
            for kc in range(KC):
                t = wpool.tile([P, HPC * HD], F32, name=f"w{kc}", tag=f"w{kc}")
                nc.sync.dma_start(t[:], wd[kc * P:(kc + 1) * P, :])
                w_sb.append(t)
            for sq in range(4):
                ssl = slice(sq * 512, (sq + 1) * 512)
                ps = [ppool.tile([P, 512], F32, name=f"pj{h}", tag=f"pj{h}")
                      for h in range(HPC)]
                for kc in range(KC):
                    xc = xpool.tile([P, 512], F32, name="xc", tag="xc")
                    nc.sync.dma_start(xc[:], xt[kc * P:(kc + 1) * P, ssl])
                    for h in range(HPC):
                        nc.tensor.matmul(
                            ps[h][:], lhsT=w_sb[kc][:, h * HD:(h + 1) * HD],
                            rhs=xc[:], start=(kc == 0), stop=(kc == KC - 1))
                for h in range(HPC):
                    dst = out_tiles[h][:, ssl]
                    m = rpool.tile([P, 512], F32, name="ropetmp", tag="ropetmp")
                    nc.vector.tensor_mul(m[:], ps[h][:], cos_sb[:, ssl])
                    rot = rpool.tile([P, 512], F32, name="roperot", tag="roperot")
                    nc.vector.tensor_scalar_mul(rot[0:64, :], ps[h][64:128, :], -1.0)
                    nc.vector.tensor_scalar_mul(rot[64:128, :], ps[h][0:64, :], 1.0)
                    rs_ = rpool.tile([P, 512], F32, name="ropesin", tag="ropesin")
                    nc.vector.tensor_mul(rs_[:], rot[:], sin_sb[:, ssl])
                    nc.vector.tensor_add(dst[:], m[:], rs_[:])
                    nc.sync.dma_start(out_dram[h * HD:(h + 1) * HD, ssl], dst)

        with tc.tile_pool(name="ppool", bufs=2, space="PSUM") as ppool:
            proj_pass(ppool, wq, qt_sb, qt_o)
            proj_pass(ppool, wk, kt_sb, kt_o)

        spool = ctx.enter_context(tc.tile_pool(name="spool", bufs=1, space="PSUM"))
        apool = ctx.enter_context(tc.tile_pool(name="apool", bufs=2, space="PSUM"))

        for h in range(HPC):
            sc_ps = spool.tile([1, S], F32, name="scps", tag="scps")
            for qt in range(16):
                qsl = slice(qt * P, (qt + 1) * P)
                E_half, rs_half = [], []
                for half in range(2):
                    hs = slice(half * 1024, (half + 1) * 1024)
                    aw = apool.tile([P, 1024], F32, name="aw", tag="aw")
                    for j in range(2):
                        nsl = slice(half * 1024 + j * 512,
                                    half * 1024 + (j + 1) * 512)
                        nc.tensor.matmul(
                            aw[:, j * 512:(j + 1) * 512],
                            lhsT=qt_sb[h][:, qsl], rhs=kt_sb[h][:, nsl],
                            start=True, stop=True)
                    mt = mpool.tile([P, 1024], F32, name="mt", tag="mt")
                    nc.sync.dma_start(mt[:], maskd[qsl, hs])
                    nc.vector.tensor_add(aw[:], aw[:], mt[:])
                    E = epool.tile([P, 1024], F32, name="E", tag="E")
                    rs = vpool.tile([P, 1], F32, name=f"rs{half}", tag=f"rs{half}")
                    nc.scalar.activation(E[:], aw[:], EXP, accum_out=rs[:])
                    E_half.append(E)
                    rs_half.append(rs)
                rtot = vpool.tile([P, 1], F32, name="rtot", tag="rtot")
                nc.vector.tensor_add(rtot[:], rs_half[0][:], rs_half[1][:])
                r = vpool.tile([P, 1], F32, name="r", tag="r")
                nc.vector.reciprocal(r[:], rtot[:])
                for hh in range(2):
                    for j in range(2):
                        osl = slice(hh * 1024 + j * 512, hh * 1024 + (j + 1) * 512)
                        nc.tensor.matmul(
                            sc_ps[:, osl], lhsT=r[:],
                            rhs=E_half[hh][:, j * 512:(j + 1) * 512],
                            start=(qt == 0), stop=(qt == 15))
            scsb = vpool.tile([1, S], F32, name="scsb", tag="scsb", bufs=1)
            nc.vector.tensor_copy(scsb[:], sc_ps[:])
            nc.sync.dma_start(scores_o[h:h + 1, :], scsb[:])
    nc.compile()
    return nc


def _build_phase2_general():
    nc = bacc.Bacc("TRN2", target_bir_lowering=False, debug=False,
                   num_devices=NCORES)
    qtd = nc.dram_tensor("qt2", [HPC * HD, S], F32, kind="ExternalInput").ap()
    ktk = nc.dram_tensor("ktk", [HPC * HD, KPAD], F32, kind="ExternalInput").ap()
    mpk = nc.dram_tensor("mpk", [HPC * KPAD, S], F32, kind="ExternalInput").ap()
    xtk = nc.dram_tensor("xtk", [H, HPC * KPAD], F32, kind="ExternalInput").ap()
    wv = nc.dram_tensor("wv", [H, HPC * HD], F32, kind="ExternalInput").ap()
    wo = nc.dram_tensor("wo", [HPC * HD, H], F32, kind="ExternalInput").ap()
    biasv = nc.dram_tensor("biasv", [P, HPC], F32, kind="ExternalInput").ap()
    out2 = nc.dram_tensor("out2", [S, H], F32, kind="ExternalOutput").ap()

    with tile.TileContext(nc) as tc, contextlib.ExitStack() as ctx:
        const = ctx.enter_context(tc.tile_pool(name="const", bufs=1))
        wvp = ctx.enter_context(tc.tile_pool(name="wvp", bufs=3))
        xkp = ctx.enter_context(tc.tile_pool(name="xkp", bufs=3))
        mpp = ctx.enter_context(tc.tile_pool(name="mpp", bufs=3))
        wop = ctx.enter_context(tc.tile_pool(name="wop", bufs=1))
        vres = ctx.enter_context(tc.tile_pool(name="vres", bufs=1))
        ores = ctx.enter_context(tc.tile_pool(name="ores", bufs=1))
        apool = ctx.enter_context(tc.tile_pool(name="apool", bufs=2))

        qt_sb = [const.tile([P, S], F32, name=f"qt{h}", tag=f"qt{h}")
                 for h in range(HPC)]
        for h in range(HPC):
            nc.sync.dma_start(qt_sb[h][:], qtd[h * HD:(h + 1) * HD, :])
        ktk_sb = [const.tile([P, KPAD], F32, name=f"ktk{h}", tag=f"ktk{h}")
                  for h in range(HPC)]
        for h in range(HPC):
            nc.sync.dma_start(ktk_sb[h][:], ktk[h * HD:(h + 1) * HD, :])
        bias_sb = const.tile([P, HPC], F32, name="biasvt", tag="biasvt")
        nc.sync.dma_start(bias_sb[:], biasv[:, :])

        v_sb = [[vres.tile([P, HD], F32, name=f"vsb{h}_{t}", tag=f"vsb{h}_{t}")
                 for t in range(2)] for h in range(HPC)]
        with tc.tile_pool(name="vps", bufs=1, space="PSUM") as vps:
            v_ps = [[vps.tile([P, HD], F32, name=f"vps{h}_{t}", tag=f"vps{h}_{t}")
                     for t in range(2)] for h in range(HPC)]
            for kc in range(KC):
                ksl = slice(kc * P, (kc + 1) * P)
                wvt = wvp.tile([P, HPC * HD], F32, name="wvt", tag="wvt")
                nc.sync.dma_start(wvt[:], wv[ksl, :])
                xkt = xkp.tile([P, HPC * KPAD], F32, name="xkt", tag="xkt")
                nc.sync.dma_start(xkt[:], xtk[ksl, :])
                for h in range(HPC):
                    for t in range(2):
                        nc.tensor.matmul(
                            v_ps[h][t][:],
                            lhsT=xkt[:, h * KPAD + t * P: h * KPAD + (t + 1) * P],
                            rhs=wvt[:, h * HD:(h + 1) * HD],
                            start=(kc == 0), stop=(kc == KC - 1))
            for h in range(HPC):
                for t in range(2):
                    nc.vector.tensor_copy(v_sb[h][t][:], v_ps[h][t][:])

        oh_sb = [ores.tile([P, S], F32, name=f"oh{h}", tag=f"oh{h}")
                 for h in range(HPC)]
        with tc.tile_pool(name="atp", bufs=1, space="PSUM") as atp:
            for h in range(HPC):
                po = atp.tile([P, S], F32, name="po", tag="po")
                for t in range(2):
                    pa = atp.tile([P, S], F32, name="pa", tag="pa")
                    for j in range(4):
                        qsl = slice(j * 512, (j + 1) * 512)
                        nc.tensor.matmul(
                            pa[:, qsl],
                            lhsT=ktk_sb[h][:, t * P:(t + 1) * P],
                            rhs=qt_sb[h][:, qsl], start=True, stop=True)
                    mp = mpp.tile([P, S], F32, name="mp", tag="mp")
                    nc.sync.dma_start(
                        mp[:], mpk[h * KPAD + t * P: h * KPAD + (t + 1) * P, :])
                    A = apool.tile([P, S], F32, name="A", tag="A")
                    nc.vector.tensor_add(A[:], pa[:], mp[:])
                    for j in range(4):
                        qsl = slice(j * 512, (j + 1) * 512)
                        nc.tensor.matmul(
                            po[:, qsl], lhsT=v_sb[h][t][:], rhs=A[:, qsl],
                            start=(t == 0), stop=(t == 1))
                nc.vector.tensor_scalar_add(oh_sb[h][:], po[:],
                                            bias_sb[:, h:h + 1])

        wps = ctx.enter_context(tc.tile_pool(name="wps", bufs=4, space="PSUM"))
        for nt in range(8):
            nsl = slice(nt * 512, (nt + 1) * 512)
            wot = [wop.tile([P, 512], F32, name=f"wot{kc}", tag=f"wot{kc}")
                   for kc in range(HPC)]
            for kc in range(HPC):
                nc.sync.dma_start(wot[kc][:], wo[kc * P:(kc + 1) * P, nsl])
            for qt in range(16):
                qsl = slice(qt * P, (qt + 1) * P)
                pw = wps.tile([P, 512], F32, name="pw", tag="pw")
                for kc in range(HPC):
                    nc.tensor.matmul(pw[:], lhsT=oh_sb[kc][:, qsl],
                                     rhs=wot[kc][:],
                                     start=(kc == 0), stop=(kc == HPC - 1))
                ow = apool.tile([P, 512], F32, name="ow", tag="ow")
                nc.vector.tensor_copy(ow[:], pw[:])
                nc.sync.dma_start(out2[qsl, nsl], ow[:])
    nc.compile()
    return nc


def _kernel_general(hidden_states, am, Wq, Wk, Wv, Wo, position_ids):
    x = np.ascontiguousarray(np.asarray(hidden_states, np.float32)[0])   # [S, H]
    Wq = np.asarray(Wq, np.float32)
    Wk = np.asarray(Wk, np.float32)
    Wv = np.asarray(Wv, np.float32)
    Wo = np.asarray(Wo, np.float32)
    pos = np.asarray(position_ids)[0]

    inv = 1.0 / (10000.0 ** (np.arange(0, HD, 2, dtype=np.float32) / HD))
    fr = pos.astype(np.float32)[:, None] * inv
    emb = np.concatenate([fr, fr], -1)
    cosT = np.ascontiguousarray(np.cos(emb).astype(np.float32).T)  # [128, S]
    sinT = np.ascontiguousarray(np.sin(emb).astype(np.float32).T)
    xT = np.ascontiguousarray(x.T)                                  # [H, S]
    scale = np.float32(1.0 / np.sqrt(HD))

    if "g1" not in _cache:
        _cache["g1"] = _build_phase1_general()
    nc1 = _cache["g1"]

    in_maps = []
    for c in range(NCORES):
        hsl = slice(c * HPC * HD, (c + 1) * HPC * HD)
        in_maps.append({
            "xt": xT,
            "wq": np.ascontiguousarray(Wq[hsl, :].T * scale),
            "wk": np.ascontiguousarray(Wk[hsl, :].T),
            "cos": cosT, "sin": sinT, "mask": am,
        })
    r1 = _run(nc1, in_maps, list(range(NCORES)))
    _cache["exec1"] = r1.exec_time_ns

    xsum = x.astype(np.float64).sum(0)                               # [H]
    in_maps2 = []
    for c in range(NCORES):
        res = r1.results[c]
        scores, qt, kt = res["scores"], res["qt"], res["kt"]
        hsl = slice(c * HPC * HD, (c + 1) * HPC * HD)
        Wv_c = Wv[hsl, :]
        ktkv = np.zeros((HPC * HD, KPAD), np.float32)
        mpkv = np.zeros((HPC * KPAD, S), np.float32)
        xtkv = np.zeros((H, HPC * KPAD), np.float32)
        for h in range(HPC):
            kept = _topk_mask_indices(scores[h])
            ktkv[h * HD:(h + 1) * HD, :NKEPT] = kt[h * HD:(h + 1) * HD, kept]
            mpkv[h * KPAD: h * KPAD + NKEPT, :] = am[:, kept].T + np.float32(1e9)
            xtkv[:, h * KPAD: h * KPAD + NKEPT] = x[kept, :].T
        vsum = (xsum @ Wv_c.astype(np.float64).T)                    # [512]
        bias = (-1e9 * vsum).astype(np.float32).reshape(HPC, HD).T   # [128, 4]
        in_maps2.append({
            "qt2": qt, "ktk": ktkv, "mpk": mpkv, "xtk": xtkv,
            "wv": np.ascontiguousarray(Wv_c.T),
            "wo": np.ascontiguousarray(Wo[:, hsl].T),
            "biasv": np.ascontiguousarray(bias),
        })

    if "g2" not in _cache:
        _cache["g2"] = _build_phase2_general()
    nc2 = _cache["g2"]
    r2 = _run(nc2, in_maps2, list(range(NCORES)))
    _cache["exec2"] = r2.exec_time_ns

    out = np.zeros((S, H), np.float32)
    for c in range(NCORES):
        out += r2.results[c]["out2"]
    return out.reshape(1, S, H)


# revision 60
# speedup vs baseline: 1.0571x; 1.0049x over previous
"""Trainium2 Bass kernel for nn_LlamaAttention_kvcache (sparse H2O attention).

Strategy (8 NeuronCores, tensor-parallel over heads, 4 heads/core):
  Phase 1 (device): q/k projections (scale folded into Wq) in bf16, RoPE,
    causal-only QK^T (lower-triangular k-slices per q-tile), exp via the
    activation engine (row sums piggybacked on accum_out; diagonal slice
    masked post-exp with gpsimd affine_select), per-head softmax column
    scores accumulated with rank-1 (1/rowsum)^T @ E matmuls.
  Host: exact top-k selection per head (matches jax.lax.top_k tie-breaking),
    gathers kept k-columns / x-rows.
  Phase 2 (device): kept-column scores + eviction decomposition
      aw_new @ v = [M o (aw + 1e9)] @ v_kept  -  1e9 * (sum_all v)
    (evicted columns contribute exactly -1e9 * v); the +1e9*step mask is
    generated on-device from kept indices (iota + tensor_scalar is_ge);
    then row-parallel o_proj in bf16. Host sums the 8 per-core partials.
  Falls back to a general (mask-as-data, fp32) path when attention_mask is
  not the standard causal mask.
"""

import contextlib
import sys

for p in ("/opt/trn_rl_repo", "/root/.axon_site/_ro/trn_rl_repo"):
    if p not in sys.path:
        sys.path.append(p)

import ml_dtypes
import numpy as np

import concourse.bacc as bacc
import concourse.mybir as mybir
import concourse.tile as tile
from concourse.bass_utils import run_bass_kernel_spmd

F32 = mybir.dt.float32
F32R = mybir.dt.float32r
F16 = mybir.dt.float16
BF16 = mybir.dt.bfloat16
NPBF = ml_dtypes.bfloat16
P = 128
S = 2048
H = 4096
NH = 32
HD = 128
NCORES = 8
HPC = NH // NCORES          # heads per core = 4
KC = H // P                 # 32 k-chunks over the 4096 contraction
KEEP = int(0.1 * S)         # 204 top-k heavy hitters
NKEPT = KEEP + 2            # + last-2 local tokens = 206
T1 = NKEPT - P              # ragged second kept-tile rows = 78
KPAD = 256                  # padded kept count (general fallback only)
EXP = mybir.ActivationFunctionType.Exp

_cache = {}


def _run(nc, in_maps, core_ids):
    return run_bass_kernel_spmd(nc, in_maps, core_ids)


def _build_phase1():
    nc = bacc.Bacc("TRN2", target_bir_lowering=False, debug=False,
                   num_devices=NCORES)
    xb = nc.dram_tensor("xb", [H, S], BF16, kind="ExternalInput").ap()
    wq = nc.dram_tensor("wq", [H, HPC * HD], BF16, kind="ExternalInput").ap()
    wk = nc.dram_tensor("wk", [H, HPC * HD], BF16, kind="ExternalInput").ap()
    cosd = nc.dram_tensor("cos", [P, S], F16, kind="ExternalInput").ap()
    sind = nc.dram_tensor("sin", [P, S], F16, kind="ExternalInput").ap()
    identd = nc.dram_tensor("ident", [P, P], F16, kind="ExternalInput").ap()
    mtaild = nc.dram_tensor("mtail", [P, 4 * 512], F16, kind="ExternalInput").ap()
    scores_o = nc.dram_tensor("scores", [HPC, S], F32,
                              kind="ExternalOutput").ap()
    qt_o = nc.dram_tensor("qt", [HPC * HD, S], F16, kind="ExternalOutput").ap()
    kt_o = nc.dram_tensor("kt", [HPC * HD, S], F16, kind="ExternalOutput").ap()

    with tile.TileContext(nc) as tc, contextlib.ExitStack() as ctx:
        const = ctx.enter_context(tc.tile_pool(name="const", bufs=1))
        wpool = ctx.enter_context(tc.tile_pool(name="wpool", bufs=1))
        xpool = ctx.enter_context(tc.tile_pool(name="xpool", bufs=10))
        qkres = ctx.enter_context(tc.tile_pool(name="qkres", bufs=1))
        rpool = ctx.enter_context(tc.tile_pool(name="rpool", bufs=2))
        epool = ctx.enter_context(tc.tile_pool(name="epool", bufs=4))
        vpool = ctx.enter_context(tc.tile_pool(name="vpool", bufs=4))

        cos_sb = const.tile([P, S], F16, name="cos", tag="cos")
        sin_sb = const.tile([P, S], F16, name="sin", tag="sin")
        ident_sb = const.tile([P, P], F16, name="ident", tag="ident")
        mtail_sb = const.tile([P, 4 * 512], F16, name="mtail", tag="mtail")
        nc.gpsimd.dma_start(ident_sb[:], identd[:, :])
        nc.gpsimd.dma_start(mtail_sb[:], mtaild[:, :])

        # resident roped q/k per head, fp16: [128 d, 2048 s]
        qt_sb = [qkres.tile([P, S], F16, name=f"qt{h}", tag=f"qt{h}")
                 for h in range(HPC)]
        kt_sb = [qkres.tile([P, S], F16, name=f"kt{h}", tag=f"kt{h}")
                 for h in range(HPC)]

        # w tiles are DMA'd lazily inside the sq=0 loop (interleaved with x
        # chunks) so the first matmul isn't stuck behind an 8MB preamble
        wq_sb = [wpool.tile([P, HPC * HD], BF16, name=f"wq{kc}", tag=f"wq{kc}")
                 for kc in range(KC)]
        wk_sb = [wpool.tile([P, HPC * HD], BF16, name=f"wk{kc}", tag=f"wk{kc}")
                 for kc in range(KC)]

        def rope(ps, out_slice, ssl):
            m = rpool.tile([P, 512], F32, name="ropetmp", tag="ropetmp")
            nc.vector.tensor_mul(m[:], ps[:], cos_sb[:, ssl])
            rot = rpool.tile([P, 512], F32, name="roperot", tag="roperot")
            nc.vector.tensor_scalar_mul(rot[0:64, :], ps[64:128, :], -1.0)
            nc.vector.tensor_scalar_mul(rot[64:128, :], ps[0:64, :], 1.0)
            rs_ = rpool.tile([P, 512], F32, name="ropesin", tag="ropesin")
            nc.vector.tensor_mul(rs_[:], rot[:], sin_sb[:, ssl])
            nc.vector.tensor_add(out_slice, m[:], rs_[:])

        # two passes (q then k) per s-quarter with double-buffered PSUM so
        # the next pass's matmuls overlap this pass's RoPE drain; x chunks
        # are resident per quarter (one DMA, reused by both passes, with
        # per-tag rolling prefetch into the next quarter)
        with tc.tile_pool(name="ppool", bufs=2, space="PSUM") as ppool:
            for sq in range(4):          # 512-wide s quarters
                ssl = slice(sq * 512, (sq + 1) * 512)
                xc_sq = []
                for w_sb, wd, out_tiles, is_q in (
                        (wq_sb, wq, qt_sb, True), (wk_sb, wk, kt_sb, False)):
                    ps = [ppool.tile([P, 512], F32, name=f"pj{h}", tag=f"pj{h}")
                          for h in range(HPC)]
                    for kc in range(KC):
                        if is_q:
                            xc = xpool.tile([P, 512], BF16, name="xc",
                                            tag=f"xc{kc}", bufs=1)
                            nc.sync.dma_start(xc[:], xb[kc * P:(kc + 1) * P, ssl])
                            xc_sq.append(xc)
                        else:
                            xc = xc_sq[kc]
                        if sq == 0:
                            nc.scalar.dma_start(w_sb[kc][:],
                                                wd[kc * P:(kc + 1) * P, :])
                            if is_q and kc == 2:
                                nc.gpsimd.dma_start(cos_sb[:], cosd[:, :])
                                nc.gpsimd.dma_start(sin_sb[:], sind[:, :])
                        for h in range(HPC):
                            nc.tensor.matmul(
                                ps[h][:], lhsT=w_sb[kc][:, h * HD:(h + 1) * HD],
                                rhs=xc[:], start=(kc == 0), stop=(kc == KC - 1))
                    for h in range(HPC):
                        rope(ps[h], out_tiles[h][:, ssl], ssl)
        for h in range(HPC):
            nc.sync.dma_start(qt_o[h * HD:(h + 1) * HD, :], qt_sb[h][:, :])
            nc.sync.dma_start(kt_o[h * HD:(h + 1) * HD, :], kt_sb[h][:, :])

        spool = ctx.enter_context(tc.tile_pool(name="spool", bufs=1, space="PSUM"))
        apool = ctx.enter_context(tc.tile_pool(name="apool", bufs=2, space="PSUM"))

        # causal attention scores per head. The strictly-future entries of
        # the diagonal 512-slice get -60000 added in PSUM via an
        # identity-weight matmul (exp then yields exact zeros), so every
        # activation is uniform with accum_out and slices pair into
        # 1024-wide activations. Score matmuls write each 512-slice to its
        # own partition of a [4, 512] one-bank accumulator and lag 2
        # q-tiles so the PE never waits on the exp/rowsum chain.
        for h in range(HPC):
            sc_ps = spool.tile([1, S], F32, name="scps", tag="scps")
            pend = []

            def flush_one():
                oqt, oE, orb, onsl = pend.pop(0)
                for s in range(onsl):
                    ksl = slice(s * 512, (s + 1) * 512)
                    nc.tensor.matmul(sc_ps[:, ksl], lhsT=orb[:],
                                     rhs=oE[:, ksl],
                                     start=(oqt == 4 * s), stop=(oqt == 15))

            for qt in range(16):
                qsl = slice(qt * P, (qt + 1) * P)
                nsl = (qt + 1 + 3) // 4      # 512-slices overlapping [0,(qt+1)*128)
                m = qt % 4                   # within-slice diagonal offset
                E = epool.tile([P, S], BF16, name="E", tag="E")
                r_parts = []
                for pr in range((nsl + 1) // 2):
                    s0 = 2 * pr
                    w = 1024 if s0 + 1 < nsl else 512
                    aw = apool.tile([P, 1024], F32, name="aw", tag="aw")
                    for k in range(w // 512):
                        s = s0 + k
                        ksl = slice(s * 512, (s + 1) * 512)
                        awsl = aw[:, k * 512:(k + 1) * 512]
                        if s == nsl - 1:
                            nc.tensor.matmul(awsl, lhsT=qt_sb[h][:, qsl],
                                             rhs=kt_sb[h][:, ksl],
                                             start=True, stop=False)
                            nc.tensor.matmul(
                                awsl, lhsT=ident_sb[:],
                                rhs=mtail_sb[:, m * 512:(m + 1) * 512],
                                start=False, stop=True)
                        else:
                            nc.tensor.matmul(awsl, lhsT=qt_sb[h][:, qsl],
                                             rhs=kt_sb[h][:, ksl],
                                             start=True, stop=True)
                    rs = vpool.tile([P, 1], F32, name=f"rs{pr}", tag=f"rs{pr}")
                    nc.scalar.activation(E[:, s0 * 512: s0 * 512 + w],
                                         aw[:, 0:w], EXP, accum_out=rs[:])
                    r_parts.append(rs)
                racc = r_parts[0]
                if len(r_parts) > 1:
                    racc = vpool.tile([P, 1], F32, name="racc", tag="racc")
                    nc.gpsimd.tensor_add(racc[:], r_parts[0][:], r_parts[1][:])
                rinv = vpool.tile([P, 1], F32, name="rinv", tag="rinv")
                nc.vector.reciprocal(rinv[:], racc[:])
                rb = vpool.tile([P, 1], BF16, name="rb", tag="rb")
                nc.gpsimd.tensor_copy(rb[:], rinv[:])
                pend.append((qt, E, rb, nsl))
                if len(pend) > 3:
                    flush_one()
            while pend:
                flush_one()
            scsb = vpool.tile([1, S], F32, name="scsb", tag="scsb", bufs=2)
            nc.vector.tensor_copy(scsb[:], sc_ps[:])
            nc.sync.dma_start(scores_o[h:h + 1, :], scsb[:])
    nc.compile()
    return nc


def _build_phase2():
    nc = bacc.Bacc("TRN2", target_bir_lowering=False, debug=False,
                   num_devices=NCORES)
    qtd = nc.dram_tensor("qt2", [HPC * HD, S], F16, kind="ExternalInput").ap()
    ktk = nc.dram_tensor("ktk", [HPC * HD, NKEPT], F16, kind="ExternalInput").ap()
    keptq = nc.dram_tensor("keptq", [P, HPC * 2], F32, kind="ExternalInput").ap()
    xtk = nc.dram_tensor("xtk", [H, HPC * NKEPT], BF16, kind="ExternalInput").ap()
    wv = nc.dram_tensor("wv", [H, HPC * HD], BF16, kind="ExternalInput").ap()
    wo = nc.dram_tensor("wo", [HPC * HD, H], BF16, kind="ExternalInput").ap()
    biasv = nc.dram_tensor("biasv", [P, HPC], F32, kind="ExternalInput").ap()
    out2 = nc.dram_tensor("out2", [S, H], BF16, kind="ExternalOutput").ap()

    with tile.TileContext(nc) as tc, contextlib.ExitStack() as ctx:
        const = ctx.enter_context(tc.tile_pool(name="const", bufs=1))
        wvp = ctx.enter_context(tc.tile_pool(name="wvp", bufs=3))
        xkp = ctx.enter_context(tc.tile_pool(name="xkp", bufs=3))
        mpp = ctx.enter_context(tc.tile_pool(name="mpp", bufs=3))
        wop = ctx.enter_context(tc.tile_pool(name="wop", bufs=2))
        vres = ctx.enter_context(tc.tile_pool(name="vres", bufs=1))
        ores = ctx.enter_context(tc.tile_pool(name="ores", bufs=1))
        apool = ctx.enter_context(tc.tile_pool(name="apool", bufs=3))

        qt_sb = [const.tile([P, S], F16, name=f"qt{h}", tag=f"qt{h}")
                 for h in range(HPC)]
        ktk_sb = [const.tile([P, NKEPT], F16, name=f"ktk{h}", tag=f"ktk{h}")
                  for h in range(HPC)]
        wo_sb = [const.tile([P, H], BF16, name=f"wo{kk}", tag=f"wo{kk}")
                 for kk in range(HPC)]
        bias_sb = const.tile([P, HPC], F32, name="biasvt", tag="biasvt")
        keptq_sb = const.tile([P, HPC * 2], F32, name="keptq", tag="keptq")
        iota_i = const.tile([P, S], mybir.dt.int32, name="iotai", tag="iotai")
        nc.gpsimd.iota(iota_i[:], pattern=[[1, S]], base=0, channel_multiplier=0)
        iota_f = const.tile([P, S], F32, name="iotaf", tag="iotaf")
        nc.vector.tensor_copy(iota_f[:], iota_i[:])

        # v projection of kept rows: v_sb[h][t] = [128 kept, 128 d].
        # The big qt/ktk const loads are queued behind the first few v-proj
        # chunk DMAs so the PE starts within a few us.
        v_sb = [[vres.tile([P if t == 0 else T1, HD], BF16,
                           name=f"vsb{h}_{t}", tag=f"vsb{h}_{t}")
                 for t in range(2)] for h in range(HPC)]
        with tc.tile_pool(name="vps", bufs=1, space="PSUM") as vps:
            v_ps = [[vps.tile([P if t == 0 else T1, HD], F32,
                              name=f"vps{h}_{t}", tag=f"vps{h}_{t}")
                     for t in range(2)] for h in range(HPC)]
            for kc in range(KC):
                ksl = slice(kc * P, (kc + 1) * P)
                wvt = wvp.tile([P, HPC * HD], BF16, name="wvt", tag="wvt")
                nc.scalar.dma_start(wvt[:], wv[ksl, :])
                xkt = xkp.tile([P, HPC * NKEPT], BF16, name="xkt", tag="xkt")
                nc.sync.dma_start(xkt[:], xtk[ksl, :])
                if kc == 2:
                    for h in range(HPC):
                        nc.gpsimd.dma_start(ktk_sb[h][:],
                                            ktk[h * HD:(h + 1) * HD, :])
                    nc.gpsimd.dma_start(bias_sb[:], biasv[:, :])
                    nc.gpsimd.dma_start(keptq_sb[:], keptq[:, :])
                if kc == 4:
                    for h in range(HPC):
                        nc.gpsimd.dma_start(qt_sb[h][:],
                                            qtd[h * HD:(h + 1) * HD, :])
                if kc == 20:
                    for kk in range(HPC):
                        nc.scalar.dma_start(wo_sb[kk][:],
                                            wo[kk * P:(kk + 1) * P, :])
                for h in range(HPC):
                    for t in range(2):
                        nc.tensor.matmul(
                            v_ps[h][t][:],
                            lhsT=xkt[:, h * NKEPT + t * P: h * NKEPT +
                                     (P if t == 0 else NKEPT)],
                            rhs=wvt[:, h * HD:(h + 1) * HD],
                            start=(kc == 0), stop=(kc == KC - 1))
            for h in range(HPC):
                for t in range(2):
                    nc.vector.tensor_copy(v_sb[h][t][:], v_ps[h][t][:])

        # step masks (+1e9 where kept_idx <= q) precomputed per (h, t)
        mp_sb = [[mpp.tile([P, S], F32, name=f"mp{h}_{t}", tag=f"mp{h}_{t}",
                           bufs=1)
                  for t in range(2)] for h in range(HPC)]
        for h in range(HPC):
            for t in range(2):
                nc.vector.tensor_scalar(
                    mp_sb[h][t][:], iota_f[:],
                    keptq_sb[:, h * 2 + t: h * 2 + t + 1],
                    1e9, mybir.AluOpType.is_ge, mybir.AluOpType.mult)

        # per-head kept attention -> ohT bf16 [128 d, 2048 q], pipelined in
        # 1024-wide q blocks; pa(t=1) matmuls overlap the t=0 A-add (gpsimd)
        oh_sb = [ores.tile([P, S], BF16, name=f"oh{h}", tag=f"oh{h}")
                 for h in range(HPC)]
        with tc.tile_pool(name="pop", bufs=2, space="PSUM") as pop, \
             tc.tile_pool(name="pap", bufs=2, space="PSUM") as pap:
            for h in range(HPC):
                for half in range(2):
                    hsl = slice(half * 1024, (half + 1) * 1024)
                    po = pop.tile([P, 1024], F32, name="po", tag="po")
                    pa_t, A_t = [], []
                    for t in range(2):
                        rows = P if t == 0 else T1
                        pa = pap.tile([P, 1024], F32, name="pa", tag="pa")
                        for j in range(2):
                            qsl = slice(half * 1024 + j * 512,
                                        half * 1024 + (j + 1) * 512)
                            nc.tensor.matmul(
                                pa[0:rows, j * 512:(j + 1) * 512],
                                lhsT=ktk_sb[h][:, t * P: P if t == 0 else NKEPT],
                                rhs=qt_sb[h][:, qsl], start=True, stop=True)
                        A = apool.tile([P, 1024], BF16, name="A", tag="A")
                        nc.vector.tensor_add(A[0:rows, :], pa[0:rows, :],
                                             mp_sb[h][t][0:rows, hsl])
                        pa_t.append(pa)
                        A_t.append(A)
                    for t in range(2):
                        rows = P if t == 0 else T1
                        for j in range(2):
                            nc.tensor.matmul(
                                po[:, j * 512:(j + 1) * 512],
                                lhsT=v_sb[h][t][:],
                                rhs=A_t[t][0:rows, j * 512:(j + 1) * 512],
                                start=(t == 0), stop=(t == 1))
                    nc.vector.tensor_scalar_add(oh_sb[h][:, hsl], po[:],
                                                bias_sb[:, h:h + 1])

        # row-parallel o_proj: out2[s, :] partial (bf16); wo fully resident,
        # one wide out2 DMA per q-tile
        wps = ctx.enter_context(tc.tile_pool(name="wps", bufs=4, space="PSUM"))
        owp = ctx.enter_context(tc.tile_pool(name="owp", bufs=2))
        for qt in range(16):
            qsl = slice(qt * P, (qt + 1) * P)
            ow = owp.tile([P, H], BF16, name="ow", tag="ow")
            for nt in range(8):
                nsl = slice(nt * 512, (nt + 1) * 512)
                pw = wps.tile([P, 512], F32, name="pw", tag="pw")
                for kc in range(HPC):
                    nc.tensor.matmul(pw[:], lhsT=oh_sb[kc][:, qsl],
                                     rhs=wo_sb[kc][:, nsl],
                                     start=(kc == 0), stop=(kc == HPC - 1))
                nc.vector.tensor_copy(ow[:, nsl], pw[:])
            nc.sync.dma_start(out2[qsl, :], ow[:, :])
    nc.compile()
    return nc


def _topk_mask_indices(scores):
    """jax.lax.top_k semantics: descending, ties -> lower index."""
    s = scores[:-2]
    idx = np.argsort(-s, kind="stable")[:KEEP]
    kept = np.concatenate([idx, [S - 2, S - 1]])
    kept.sort()
    return kept.astype(np.int64)


def _is_causal(am):
    if am.shape != (S, S):
        return False
    k = np.arange(S)
    row0 = np.where(k <= 0, 0.0, -1e9).astype(np.float32)
    if not np.array_equal(am[0], row0):
        return False
    tri = np.where(k[:, None] >= k[None, :], 0.0, -1e9).astype(np.float32)
    return np.array_equal(am, tri)


def kernel(hidden_states, attention_mask, Wq, Wk, Wv, Wo, position_ids):
    am = np.ascontiguousarray(np.asarray(attention_mask, np.float32)[0, 0])
    if not _is_causal(am):
        return _kernel_general(hidden_states, am, Wq, Wk, Wv, Wo, position_ids)

    x = np.ascontiguousarray(np.asarray(hidden_states, np.float32)[0])   # [S, H]
    Wq = np.asarray(Wq, np.float32)
    Wk = np.asarray(Wk, np.float32)
    Wv = np.asarray(Wv, np.float32)
    Wo = np.asarray(Wo, np.float32)
    pos = np.asarray(position_ids)[0]

    inv = 1.0 / (10000.0 ** (np.arange(0, HD, 2, dtype=np.float32) / HD))
    fr = pos.astype(np.float32)[:, None] * inv
    emb = np.concatenate([fr, fr], -1)
    cosT = np.ascontiguousarray(np.cos(emb).astype(np.float16).T)  # [128, S]
    sinT = np.ascontiguousarray(np.sin(emb).astype(np.float16).T)
    xT = np.ascontiguousarray(x.T)                                  # [H, S]
    xTb = xT.astype(NPBF)
    identv = np.eye(P, dtype=np.float16)
    jj = np.arange(512)[None, :]
    pp = np.arange(P)[:, None]
    mtailv = np.concatenate(
        [np.where(jj > m * P + pp, np.float16(-60000), np.float16(0))
         for m in range(4)], axis=1)                                # [128, 2048]
    scale = np.float32(1.0 / np.sqrt(HD))

    if "p1" not in _cache:
        _cache["p1"] = _build_phase1()
    nc1 = _cache["p1"]

    in_maps = []
    for c in range(NCORES):
        hsl = slice(c * HPC * HD, (c + 1) * HPC * HD)
        in_maps.append({
            "xb": xTb,
            "wq": np.ascontiguousarray((Wq[hsl, :].T * scale).astype(NPBF)),
            "wk": np.ascontiguousarray(Wk[hsl, :].T.astype(NPBF)),
            "cos": cosT, "sin": sinT, "ident": identv, "mtail": mtailv,
        })
    r1 = _run(nc1, in_maps, list(range(NCORES)))
    _cache["exec1"] = r1.exec_time_ns

    # host: top-k + gathers
    xsum = x.astype(np.float64).sum(0)                               # [H]
    in_maps2 = []
    for c in range(NCORES):
        res = r1.results[c]
        scores = res["scores"]
        qt, kt = res["qt"], res["kt"]
        hsl = slice(c * HPC * HD, (c + 1) * HPC * HD)
        Wv_c = Wv[hsl, :]
        ktkv = np.zeros((HPC * HD, NKEPT), np.float16)
        keptqv = np.full((P, HPC * 2), 1e9, np.float32)
        xtkv = np.zeros((H, HPC * NKEPT), NPBF)
        for h in range(HPC):
            kept = _topk_mask_indices(scores[h])
            ktkv[h * HD:(h + 1) * HD, :] = kt[h * HD:(h + 1) * HD, kept]
            kf = kept.astype(np.float32)
            keptqv[:T1, h * 2 + 1] = kf[P:]
            keptqv[:, h * 2] = kf[:P]
            xtkv[:, h * NKEPT:(h + 1) * NKEPT] = x[kept, :].T.astype(NPBF)
        vsum = (xsum @ Wv_c.astype(np.float64).T)                    # [512]
        bias = (-1e9 * vsum).astype(np.float32).reshape(HPC, HD).T   # [128, 4]
        in_maps2.append({
            "qt2": qt, "ktk": ktkv, "keptq": keptqv, "xtk": xtkv,
            "wv": np.ascontiguousarray(Wv_c.T.astype(NPBF)),
            "wo": np.ascontiguousarray(Wo[:, hsl].T.astype(NPBF)),
            "biasv": np.ascontiguousarray(bias),
        })

    if "p2" not in _cache:
        _cache["p2"] = _build_phase2()
    nc2 = _cache["p2"]
    r2 = _run(nc2, in_maps2, list(range(NCORES)))
    _cache["exec2"] = r2.exec_time_ns

    out = np.zeros((S, H), np.float32)
    for c in range(NCORES):
        out += r2.results[c]["out2"].astype(np.float32)
    return out.reshape(1, S, H)


# ───────────────────────── general-mask fallback (fp32) ─────────────────────


def _build_phase1_general():
    nc = bacc.Bacc("TRN2", target_bir_lowering=False, debug=False,
                   num_devices=NCORES)
    xt = nc.dram_tensor("xt", [H, S], F32, kind="ExternalInput").ap()
    wq = nc.dram_tensor("wq", [H, HPC * HD], F32, kind="ExternalInput").ap()
    wk = nc.dram_tensor("wk", [H, HPC * HD], F32, kind="ExternalInput").ap()
    cosd = nc.dram_tensor("cos", [P, S], F32, kind="ExternalInput").ap()
    sind = nc.dram_tensor("sin", [P, S], F32, kind="ExternalInput").ap()
    maskd = nc.dram_tensor("mask", [S, S], F32, kind="ExternalInput").ap()
    scores_o = nc.dram_tensor("scores", [HPC, S], F32, kind="ExternalOutput").ap()
    qt_o = nc.dram_tensor("qt", [HPC * HD, S], F32, kind="ExternalOutput").ap()
    kt_o = nc.dram_tensor("kt", [HPC * HD, S], F32, kind="ExternalOutput").ap()

    with tile.TileContext(nc) as tc, contextlib.ExitStack() as ctx:
        const = ctx.enter_context(tc.tile_pool(name="const", bufs=1))
        wpool = ctx.enter_context(tc.tile_pool(name="wpool", bufs=1))
        xpool = ctx.enter_context(tc.tile_pool(name="xpool", bufs=4))
        qkres = ctx.enter_context(tc.tile_pool(name="qkres", bufs=1))
        rpool = ctx.enter_context(tc.tile_pool(name="rpool", bufs=2))
        mpool = ctx.enter_context(tc.tile_pool(name="mpool", bufs=3))
        epool = ctx.enter_context(tc.tile_pool(name="epool", bufs=3))
        vpool = ctx.enter_context(tc.tile_pool(name="vpool", bufs=2))

        cos_sb = const.tile([P, S], F32, name="cos", tag="cos")
        sin_sb = const.tile([P, S], F32, name="sin", tag="sin")
        nc.sync.dma_start(cos_sb[:], cosd[:, :])
        nc.sync.dma_start(sin_sb[:], sind[:, :])

        qt_sb = [qkres.tile([P, S], F32, name=f"qt{h}", tag=f"qt{h}")
                 for h in range(HPC)]
        kt_sb = [qkres.tile([P, S], F32, name=f"kt{h}", tag=f"kt{h}")
                 for h in range(HPC)]

        def proj_pass(ppool, wd, out_tiles, out_dram):
            w_sb = []
            for kc in range(KC):
                t = wpool.tile([P, HPC * HD], F32, name=f"w{kc}", tag=f"w{kc}")
                nc.sync.dma_start(t[:], wd[kc * P:(kc + 1) * P, :])
                w_sb.append(t)
            for sq in range(4):
                ssl = slice(sq * 512, (sq + 1) * 512)
                ps = [ppool.tile([P, 512], F32, name=f"pj{h}", tag=f"pj{h}")
                      for h in range(HPC)]
                for kc in range(KC):
                    xc = xpool.tile([P, 512], F32, name="xc", tag="xc")
                    nc.sync.dma_start(xc[:], xt[kc * P:(kc + 1) * P, ssl])
                    for h in range(HPC):
                        nc.tensor.matmul(
                            ps[h][:], lhsT=w_sb[kc][:, h * HD:(h + 1) * HD],
                            rhs=xc[:], start=(kc == 0), stop=(kc == KC - 1))
                for h in range(HPC):
                    dst = out_tiles[h][:, ssl]
                    m = rpool.tile([P, 512], F32, name="ropetmp", tag="ropetmp")
                    nc.vector.tensor_mul(m[:], ps[h][:], cos_sb[:, ssl])
                    rot = rpool.tile([P, 512], F32, name="roperot", tag="roperot")
                    nc.vector.tensor_scalar_mul(rot[0:64, :], ps[h][64:128, :], -1.0)
                    nc.vector.tensor_scalar_mul(rot[64:128, :], ps[h][0:64, :], 1.0)
                    rs_ = rpool.tile([P, 512], F32, name="ropesin", tag="ropesin")
                    nc.vector.tensor_mul(rs_[:], rot[:], sin_sb[:, ssl])
                    nc.vector.tensor_add(dst[:], m[:], rs_[:])
                    nc.sync.dma_start(out_dram[h * HD:(h + 1) * HD, ssl], dst)

        with tc.tile_pool(name="ppool", bufs=2, space="PSUM") as ppool:
            proj_pass(ppool, wq, qt_sb, qt_o)
            proj_pass(ppool, wk, kt_sb, kt_o)

        spool = ctx.enter_context(tc.tile_pool(name="spool", bufs=1, space="PSUM"))
        apool = ctx.enter_context(tc.tile_pool(name="apool", bufs=2, space="PSUM"))

        for h in range(HPC):
            sc_ps = spool.tile([1, S], F32, name="scps", tag="scps")
            for qt in range(16):
                qsl = slice(qt * P, (qt + 1) * P)
                E_half, rs_half = [], []
                for half in range(2):
                    hs = slice(half * 1024, (half + 1) * 1024)
                    aw = apool.tile([P, 1024], F32, name="aw", tag="aw")
                    for j in range(2):
                        nsl = slice(half * 1024 + j * 512,
                                    half * 1024 + (j + 1) * 512)
                        nc.tensor.matmul(
                            aw[:, j * 512:(j + 1) * 512],
                            lhsT=qt_sb[h][:, qsl], rhs=kt_sb[h][:, nsl],
                            start=True, stop=True)
                    mt = mpool.tile([P, 1024], F32, name="mt", tag="mt")
                    nc.sync.dma_start(mt[:], maskd[qsl, hs])
                    nc.vector.tensor_add(aw[:], aw[:], mt[:])
                    E = epool.tile([P, 1024], F32, name="E", tag="E")
                    rs = vpool.tile([P, 1], F32, name=f"rs{half}", tag=f"rs{half}")
                    nc.scalar.activation(E[:], aw[:], EXP, accum_out=rs[:])
                    E_half.append(E)
                    rs_half.append(rs)
                rtot = vpool.tile([P, 1], F32, name="rtot", tag="rtot")
                nc.vector.tensor_add(rtot[:], rs_half[0][:], rs_half[1][:])
                r = vpool.tile([P, 1], F32, name="r", tag="r")
                nc.vector.reciprocal(r[:], rtot[:])
                for hh in range(2):
                    for j in range(2):
                        osl = slice(hh * 1024 + j * 512, hh * 1024 + (j + 1) * 512)
                        nc.tensor.matmul(
                            sc_ps[:, osl], lhsT=r[:],
                            rhs=E_half[hh][:, j * 512:(j + 1) * 512],
                            start=(qt == 0), stop=(qt == 15))
            scsb = vpool.tile([1, S], F32, name="scsb", tag="scsb", bufs=1)
            nc.vector.tensor_copy(scsb[:], sc_ps[:])
            nc.sync.dma_start(scores_o[h:h + 1, :], scsb[:])
    nc.compile()
    return nc


def _build_phase2_general():
    nc = bacc.Bacc("TRN2", target_bir_lowering=False, debug=False,
                   num_devices=NCORES)
    qtd = nc.dram_tensor("qt2", [HPC * HD, S], F32, kind="ExternalInput").ap()
    ktk = nc.dram_tensor("ktk", [HPC * HD, KPAD], F32, kind="ExternalInput").ap()
    mpk = nc.dram_tensor("mpk", [HPC * KPAD, S], F32, kind="ExternalInput").ap()
    xtk = nc.dram_tensor("xtk", [H, HPC * KPAD], F32, kind="ExternalInput").ap()
    wv = nc.dram_tensor("wv", [H, HPC * HD], F32, kind="ExternalInput").ap()
    wo = nc.dram_tensor("wo", [HPC * HD, H], F32, kind="ExternalInput").ap()
    biasv = nc.dram_tensor("biasv", [P, HPC], F32, kind="ExternalInput").ap()
    out2 = nc.dram_tensor("out2", [S, H], F32, kind="ExternalOutput").ap()

    with tile.TileContext(nc) as tc, contextlib.ExitStack() as ctx:
        const = ctx.enter_context(tc.tile_pool(name="const", bufs=1))
        wvp = ctx.enter_context(tc.tile_pool(name="wvp", bufs=3))
        xkp = ctx.enter_context(tc.tile_pool(name="xkp", bufs=3))
        mpp = ctx.enter_context(tc.tile_pool(name="mpp", bufs=3))
        wop = ctx.enter_context(tc.tile_pool(name="wop", bufs=1))
        vres = ctx.enter_context(tc.tile_pool(name="vres", bufs=1))
        ores = ctx.enter_context(tc.tile_pool(name="ores", bufs=1))
        apool = ctx.enter_context(tc.tile_pool(name="apool", bufs=2))

        qt_sb = [const.tile([P, S], F32, name=f"qt{h}", tag=f"qt{h}")
                 for h in range(HPC)]
        for h in range(HPC):
            nc.sync.dma_start(qt_sb[h][:], qtd[h * HD:(h + 1) * HD, :])
        ktk_sb = [const.tile([P, KPAD], F32, name=f"ktk{h}", tag=f"ktk{h}")
                  for h in range(HPC)]
        for h in range(HPC):
            nc.sync.dma_start(ktk_sb[h][:], ktk[h * HD:(h + 1) * HD, :])
        bias_sb = const.tile([P, HPC], F32, name="biasvt", tag="biasvt")
        nc.sync.dma_start(bias_sb[:], biasv[:, :])

        v_sb = [[vres.tile([P, HD], F32, name=f"vsb{h}_{t}", tag=f"vsb{h}_{t}")
                 for t in range(2)] for h in range(HPC)]
        with tc.tile_pool(name="vps", bufs=1, space="PSUM") as vps:
            v_ps = [[vps.tile([P, HD], F32, name=f"vps{h}_{t}", tag=f"vps{h}_{t}")
                     for t in range(2)] for h in range(HPC)]
            for kc in range(KC):
                ksl = slice(kc * P, (kc + 1) * P)
                wvt = wvp.tile([P, HPC * HD], F32, name="wvt", tag="wvt")
                nc.sync.dma_start(wvt[:], wv[ksl, :])
                xkt = xkp.tile([P, HPC * KPAD], F32, name="xkt", tag="xkt")
                nc.sync.dma_start(xkt[:], xtk[ksl, :])
                for h in range(HPC):
                    for t in range(2):
                        nc.tensor.matmul(
                            v_ps[h][t][:],
                            lhsT=xkt[:, h * KPAD + t * P: h * KPAD + (t + 1) * P],
                            rhs=wvt[:, h * HD:(h + 1) * HD],
                            start=(kc == 0), stop=(kc == KC - 1))
            for h in range(HPC):
                for t in range(2):
                    nc.vector.tensor_copy(v_sb[h][t][:], v_ps[h][t][:])

        oh_sb = [ores.tile([P, S], F32, name=f"oh{h}", tag=f"oh{h}")
                 for h in range(HPC)]
        with tc.tile_pool(name="atp", bufs=1, space="PSUM") as atp:
            for h in range(HPC):
                po = atp.tile([P, S], F32, name="po", tag="po")
                for t in range(2):
                    pa = atp.tile([P, S], F32, name="pa", tag="pa")
                    for j in range(4):
                        qsl = slice(j * 512, (j + 1) * 512)
                        nc.tensor.matmul(
                            pa[:, qsl],
                            lhsT=ktk_sb[h][:, t * P:(t + 1) * P],
                            rhs=qt_sb[h][:, qsl], start=True, stop=True)
                    mp = mpp.tile([P, S], F32, name="mp", tag="mp")
                    nc.sync.dma_start(
                        mp[:], mpk[h * KPAD + t * P: h * KPAD + (t + 1) * P, :])
                    A = apool.tile([P, S], F32, name="A", tag="A")
                    nc.vector.tensor_add(A[:], pa[:], mp[:])
                    for j in range(4):
                        qsl = slice(j * 512, (j + 1) * 512)
                        nc.tensor.matmul(
                            po[:, qsl], lhsT=v_sb[h][t][:], rhs=A[:, qsl],
                            start=(t == 0), stop=(t == 1))
                nc.vector.tensor_scalar_add(oh_sb[h][:], po[:],
                                            bias_sb[:, h:h + 1])

        wps = ctx.enter_context(tc.tile_pool(name="wps", bufs=4, space="PSUM"))
        for nt in range(8):
            nsl = slice(nt * 512, (nt + 1) * 512)
            wot = [wop.tile([P, 512], F32, name=f"wot{kc}", tag=f"wot{kc}")
                   for kc in range(HPC)]
            for kc in range(HPC):
                nc.sync.dma_start(wot[kc][:], wo[kc * P:(kc + 1) * P, nsl])
            for qt in range(16):
                qsl = slice(qt * P, (qt + 1) * P)
                pw = wps.tile([P, 512], F32, name="pw", tag="pw")
                for kc in range(HPC):
                    nc.tensor.matmul(pw[:], lhsT=oh_sb[kc][:, qsl],
                                     rhs=wot[kc][:],
                                     start=(kc == 0), stop=(kc == HPC - 1))
                ow = apool.tile([P, 512], F32, name="ow", tag="ow")
                nc.vector.tensor_copy(ow[:], pw[:])
                nc.sync.dma_start(out2[qsl, nsl], ow[:])
    nc.compile()
    return nc


def _kernel_general(hidden_states, am, Wq, Wk, Wv, Wo, position_ids):
    x = np.ascontiguousarray(np.asarray(hidden_states, np.float32)[0])   # [S, H]
    Wq = np.asarray(Wq, np.float32)
    Wk = np.asarray(Wk, np.float32)
    Wv = np.asarray(Wv, np.float32)
    Wo = np.asarray(Wo, np.float32)
    pos = np.asarray(position_ids)[0]

    inv = 1.0 / (10000.0 ** (np.arange(0, HD, 2, dtype=np.float32) / HD))
    fr = pos.astype(np.float32)[:, None] * inv
    emb = np.concatenate([fr, fr], -1)
    cosT = np.ascontiguousarray(np.cos(emb).astype(np.float32).T)  # [128, S]
    sinT = np.ascontiguousarray(np.sin(emb).astype(np.float32).T)
    xT = np.ascontiguousarray(x.T)                                  # [H, S]
    scale = np.float32(1.0 / np.sqrt(HD))

    if "g1" not in _cache:
        _cache["g1"] = _build_phase1_general()
    nc1 = _cache["g1"]

    in_maps = []
    for c in range(NCORES):
        hsl = slice(c * HPC * HD, (c + 1) * HPC * HD)
        in_maps.append({
            "xt": xT,
            "wq": np.ascontiguousarray(Wq[hsl, :].T * scale),
            "wk": np.ascontiguousarray(Wk[hsl, :].T),
            "cos": cosT, "sin": sinT, "mask": am,
        })
    r1 = _run(nc1, in_maps, list(range(NCORES)))
    _cache["exec1"] = r1.exec_time_ns

    xsum = x.astype(np.float64).sum(0)                               # [H]
    in_maps2 = []
    for c in range(NCORES):
        res = r1.results[c]
        scores, qt, kt = res["scores"], res["qt"], res["kt"]
        hsl = slice(c * HPC * HD, (c + 1) * HPC * HD)
        Wv_c = Wv[hsl, :]
        ktkv = np.zeros((HPC * HD, KPAD), np.float32)
        mpkv = np.zeros((HPC * KPAD, S), np.float32)
        xtkv = np.zeros((H, HPC * KPAD), np.float32)
        for h in range(HPC):
            kept = _topk_mask_indices(scores[h])
            ktkv[h * HD:(h + 1) * HD, :NKEPT] = kt[h * HD:(h + 1) * HD, kept]
            mpkv[h * KPAD: h * KPAD + NKEPT, :] = am[:, kept].T + np.float32(1e9)
            xtkv[:, h * KPAD: h * KPAD + NKEPT] = x[kept, :].T
        vsum = (xsum @ Wv_c.astype(np.float64).T)                    # [512]
        bias = (-1e9 * vsum).astype(np.float32).reshape(HPC, HD).T   # [128, 4]
        in_maps2.append({
            "qt2": qt, "ktk": ktkv, "mpk": mpkv, "xtk": xtkv,
            "wv": np.ascontiguousarray(Wv_c.T),
            "wo": np.ascontiguousarray(Wo[:, hsl].T),
            "biasv": np.ascontiguousarray(bias),
        })

    if "g2" not in _cache:
        _cache["g2"] = _build_phase2_general()
    nc2 = _cache["g2"]
    r2 = _run(nc2, in_maps2, list(range(NCORES)))
    _cache["exec2"] = r2.exec_time_ns

    out = np.zeros((S, H), np.float32)
    for c in range(NCORES):
        out += r2.results[c]["out2"]
    return out.reshape(1, S, H)
